# revision 43
# baseline (speedup 1.0000x reference)
"""Trainium2 Bass kernel for nn_AttentionDecoder_82738249990894 (B=4, T=1024,
C=1024, H=16, D=64, F=4096, L=4, vocab 64+1 outputs).

Sharding: sequence-split data parallel over 8 cores.  Core c handles batch
b = c//2, sequence half = c%2.  Balanced causal split: half0 owns global
128-row blocks [0,1,6,7], half1 owns [2,3,4,5] (equal attention work: both
see 18 causal k-tiles).  Per layer the pair exchanges rmsnorm'd activations
(bf16, pairwise AllGather, ~1MB) and each core recomputes k/v for all 1024
tokens locally.  No other communication.

SPMD uniformity: one graph runs on all 8 cores, so the key/value strip is
kept in GLOBAL token order (the AllGather return scatters both pair slots
to fixed global positions) and every local q-tile j computes scores against
the union visibility vis_u=[3,4,7,8] k-tiles; per-core 0/1 masks (input
data) encode causality and half-dependent visibility.

Matmul dtypes: fp8e4m3 with DoubleRow perf mode (2 k-tiles per pass, 2x
PE throughput) for the qkv/Wo/W1/W2/AV matmuls; weights are pre-scaled
(x64, x512 for Wq) into fp8's dynamic range on the host and the inverse
scale is folded into the psum->sbuf copy or activation that follows.
Scores stay bf16 (64-deep contraction, DoubleRow inapplicable).
Residual x stays fp32.  Softmax skips max-subtraction (scores are O(1);
fp32 psum exp is safe) and gets denominators free via a ones-column
appended to v; normalization is deferred to after the AV matmul.
"""
import os
import sys
import types

sys.path.insert(0, "/opt/trn_rl_repo")

import numpy as np
import ml_dtypes

import antenv

if not hasattr(antenv, "axon_hooks"):
    _mod = types.ModuleType("antenv.axon_hooks")
    _mod._hook = None
    _mod.set_axon_ntff_profile_hook = lambda h: setattr(_mod, "_hook", h)
    _mod.get_axon_ntff_profile_hook = lambda: _mod._hook
    sys.modules["antenv.axon_hooks"] = _mod
    antenv.axon_hooks = _mod
    try:
        from trn_agent_boot.trn_boot import _ntff_profile_via_ctypes

        _mod.set_axon_ntff_profile_hook(
            _ntff_profile_via_ctypes("/opt/axon/libaxon_pjrt.so")
        )
    except Exception:
        pass

import concourse.bass as bass
import concourse.mybir as mybir
import concourse.tile as tile
from concourse import bass_utils

bass_utils.upload_artifacts = lambda tmpdir: "local://" + tmpdir
try:
    from concourse import tile_utils as _tu

    _tu.max_sbuf_usage = 206 * 1024
except Exception:
    pass

F32 = mybir.dt.float32
F32R = mybir.dt.float32r
BF16 = mybir.dt.bfloat16
F8 = mybir.dt.float8e4
DR = mybir.MatmulPerfMode.DoubleRow
AF = mybir.ActivationFunctionType
OP = mybir.AluOpType
AX = mybir.AxisListType

SW = 64.0     # fp8 weight pre-scale (Wk/Wv/Wo/W1/W2)
SWQ = 512.0   # Wq pre-scale (D^-0.5 folded in makes it 8x smaller)

B, T, C, H, D, F, L = 4, 1024, 1024, 16, 64, 4096, 4
VOCAB, OUT = 64, 65
EPS = float(np.finfo(np.float32).eps)
RG = [[0, 1], [2, 3], [4, 5], [6, 7]]
OWN_BLOCKS = {0: [0, 1, 6, 7], 1: [2, 3, 4, 5]}
VIS_U = [3, 4, 7, 8]          # union visible k-tiles per local q-tile
N_MASK = 3                    # last 3 visible slots carry a mask

_wsplit_ctr = [0]


def _split_sync_waits(nc):
    """This walrus build allows one sync-wait per instruction; hoist extras
    onto injected same-engine NoOps."""
    for f in nc.m.functions:
        for bb in f.blocks:
            out = []
            changed = False
            for inst in bb.instructions:
                si = getattr(inst, "sync_info", None)
                if si is not None and si.on_wait is not None and len(si.on_wait) > 1:
                    waits = list(si.on_wait)
                    for w in waits[:-1]:
                        _wsplit_ctr[0] += 1
                        n = mybir.InstNoOp(
                            name=f"WSPLIT-{_wsplit_ctr[0]}", ins=[], outs=[]
                        )
                        n.engine = inst.engine
                        n.sync_info = mybir.SyncInfo(on_wait=[w], on_update=[])
                        out.append(n)
                    inst.sync_info = mybir.SyncInfo(
                        on_wait=[waits[-1]], on_update=list(si.on_update)
                    )
                    changed = True
                out.append(inst)
            if changed:
                bb.instructions[:] = out


def build_graph():
    nc = bass.Bass()
    dp = nc.declare_dram_parameter
    onehot_ext = dp("onehot_t", [OUT, 512], F32R, isOutput=False)
    pos_ext = dp("pos_fm", [128, 8, 512], BF16, isOutput=False)
    aug_ext = dp("aug_table", [OUT, 8, 128], F32R, isOutput=False)
    mask_ext = dp("masks", [128, 8, 512], BF16, isOutput=False)
    onescol_ext = dp("ones_col", [128, 1], BF16, isOutput=False)
    onesrow_ext = dp("ones_row", [1, 128], F32R, isOutput=False)
    onesrowb_ext = dp("ones_row_bf", [1, 128], BF16, isOutput=False)
    wq_ext = dp("Wq_arr", [L, 8, 128, 8, 128], F8, isOutput=False)
    wk_ext = dp("Wk_arr", [L, 8, 128, 8, 128], F8, isOutput=False)
    wv_ext = dp("Wv_arr", [L, 128, 2, 8, 512], F8, isOutput=False)
    wo_ext = dp("Wo_arr", [L, 8, 128, 8, 128], F8, isOutput=False)
    w1_ext = dp("W1_arr", [L, 32, 128, 8, 128], F8, isOutput=False)
    w2_ext = dp("W2_arr", [L, 16, 128, 2, 8, 128], F8, isOutput=False)
    bo_ext = dp("bo_fm", [128, L, 8, 1], F32, isOutput=False)
    b1_ext = dp("b1_fm", [128, L, 32, 1], F32, isOutput=False)
    b2_ext = dp("b2_fm", [128, L, 8, 1], F32, isOutput=False)
    lmw_ext = dp("lmW_arr", [128, 8, OUT], BF16, isOutput=False)
    lmb_ext = dp("lmb_bc", [128, OUT], F32, isOutput=False)
    out_ext = dp("out", [512, OUT], F32, isOutput=True)
    debug = bool(int(os.environ.get("KERNEL_DEBUG", "0")))
    if debug:
        dbg_h = dp("dbg_h", [128, 8, 1024], F8, isOutput=True)
        dbg_k = dp("dbg_k", [128, 8, 1024], BF16, isOutput=True)
        dbg_q = dp("dbg_q", [128, 8, 512], BF16, isOutput=True)
        dbg_v = dp("dbg_v", [128, 8, 16, 66], F8, isOutput=True)
        dbg_o = dp("dbg_o", [128, 8, 512], F8, isOutput=True)
        dbg_u = dp("dbg_u", [128, 4, 512], F8, isOutput=True)
        dbg_x2 = dp("dbg_x2", [128, 8, 512], F32, isOutput=True)

    with tile.TileContext(nc) as tc:
        nc_lp = nc.allow_low_precision(reason="bf16 attention path is intentional")
        nc_lp.__enter__()
        with (
            tc.tile_pool(name="persist", bufs=1) as pp,
            tc.tile_pool(name="scratch", bufs=2) as sp,
            tc.tile_pool(name="wqk", bufs=3) as wqkp,
            tc.tile_pool(name="w512", bufs=3) as w512p,
            tc.tile_pool(name="w2p", bufs=5) as w2p,
            tc.tile_pool(name="bigp", bufs=1) as bigp,
            tc.tile_pool(name="wvp", bufs=1) as wvp,
            tc.tile_pool(name="ps512", bufs=2, space="PSUM") as ps512,
            tc.tile_pool(name="ps128", bufs=2, space="PSUM") as ps128,
            tc.tile_pool(name="ps128o", bufs=2, space="PSUM") as ps128o,
            tc.tile_pool(name="dram", bufs=2, space="DRAM") as dram,
        ):
            # ---- constants ----
            ones_col = pp.tile([128, 1], BF16)
            ones_row = pp.tile([1, 128], F32R)
            ones_row_bf = pp.tile([1, 128], BF16)
            aug_sb = pp.tile([OUT, 8, 128], F32R)
            onehot_sb = pp.tile([OUT, 512], F32R)
            mask_sb = pp.tile([128, 8, 512], BF16)
            lmw_sb = pp.tile([128, 8, OUT], BF16)
            lmb_sb = pp.tile([128, OUT], F32)
            bo_sb = pp.tile([128, L, 8, 1], F32)
            b1_sb = pp.tile([128, L, 32, 1], F32)
            b2_sb = pp.tile([128, L, 8, 1], F32)
            nc.sync.dma_start(aug_sb[:], aug_ext[:])
            nc.sync.dma_start(onehot_sb[:], onehot_ext[:])
            nc.sync.dma_start(ones_col[:], onescol_ext[:])
            nc.sync.dma_start(ones_row[:], onesrow_ext[:])
            nc.sync.dma_start(ones_row_bf[:], onesrowb_ext[:])

            # tiny warm-up collective: absorbs first-use CC latency while
            # the tensor engine runs the embedding
            cw_in = dram.tile([1, 64], F32, tag="cwi", name="cwi")
            cw_out = dram.tile([2, 64], F32, tag="cwo", name="cwo")
            cw_sb = pp.tile([1, 64], F32)
            nc.gpsimd.memset(cw_sb[:], 0.0)
            nc.sync.dma_start(cw_in[:], cw_sb[:])
            nc.gpsimd.collective_compute(
                "AllGather", OP.bypass,
                ins=[cw_in[:].opt()],
                outs=[cw_out[:].opt()],
                replica_groups=RG,
            )

            nc.sync.dma_start(mask_sb[:], mask_ext[:])
            nc.sync.dma_start(lmw_sb[:], lmw_ext[:])
            nc.sync.dma_start(lmb_sb[:], lmb_ext[:])
            nc.sync.dma_start(bo_sb[:], bo_ext[:])
            nc.sync.dma_start(b1_sb[:], b1_ext[:])
            nc.sync.dma_start(b2_sb[:], b2_ext[:])

            eps_sb = pp.tile([128, 1], F32)
            nc.gpsimd.memset(eps_sb[:], EPS)

            # ---- persistent activations ----
            x_sb = pp.tile([128, 8, 512], F32)       # residual (feature-major)
            h_own = pp.tile([128, 8, 512], F8)       # norm'd own tokens
            h_str = pp.tile([128, 8, 1024], F8)      # norm'd pair, global order
            q_sb = pp.tile([128, 8, 512], BF16)      # [2h*64, hp, local t]
            k_sb = pp.tile([128, 8, 1024], BF16)     # [2h*64, hp, global t]
            # inner dim padded 65->66 so every fp8 slice lands on an even
            # byte offset (odd SBUF offsets break engine reads)
            v_sb = pp.tile([128, 8, 16, 66], F8)     # [tk, tkt, head, d+1+pad]
            o_sb = pp.tile([128, 8, 512], BF16)      # attn out, unnormalized
            o_f8 = pp.tile([128, 8, 512], F8)        # normalized attn out
            # pos and per-layer h2 share one big slot (disjoint lifetimes)
            pos_sb = bigp.tile([128, 8, 512], BF16, tag="big", name="pos")
            nc.sync.dma_start(pos_sb[:], pos_ext[:])

            # ---- embedding: x = onehot @ aug_table + pos ----
            sc_emb = nc.named_scope("emb"); sc_emb.__enter__()
            for ct in range(8):
                emb_ps = ps512.tile([128, 512], F32, tag="p5", name=f"emb{ct}")
                nc.tensor.matmul(emb_ps[:], aug_sb[:, ct, :], onehot_sb[:],
                                 start=True, stop=True)
                nc.vector.tensor_add(x_sb[:, ct, :], emb_ps[:], pos_sb[:, ct, :])

            sc_emb.__exit__(None, None, None)

            def rms_rbc(tag):
                ssum = ps512.tile([128, 512], F32, tag="p5", name=f"ss{tag}")
                for ct in range(8):
                    xsq = sp.tile([128, 512], BF16, tag="xsq", name=f"xq{tag}{ct}")
                    if ct % 2 == 0:
                        nc.scalar.activation(xsq[:], x_sb[:, ct, :], AF.Square)
                    else:
                        nc.vector.tensor_tensor(xsq[:], x_sb[:, ct, :],
                                                x_sb[:, ct, :], OP.mult)
                    nc.tensor.matmul(ssum[:1, :], ones_col[:], xsq[:],
                                     start=(ct == 0), stop=(ct == 7))
                lnv = sp.tile([1, 512], F32, tag="lnv", name=f"lv{tag}", bufs=1)
                nc.scalar.activation(lnv[:], ssum[:1, :], AF.Ln,
                                     bias=eps_sb[:1, :], scale=1.0 / C)
                rstd = sp.tile([1, 512], F32R, tag="sqv", name=f"sv{tag}",
                               bufs=1)
                nc.scalar.activation(rstd[:], lnv[:], AF.Exp, scale=-0.5)
                rbc = ps512.tile([128, 512], F32, tag="p5", name=f"rb{tag}")
                nc.tensor.matmul(rbc[:], ones_row[:], rstd[:], start=True,
                                 stop=True)
                return rbc

            for l in range(L):
                sc_n1 = nc.named_scope(f"n1.{l}"); sc_n1.__enter__()
                # ===== norm1 -> h_own =====
                rbc = rms_rbc(f"a{l}")
                for ct in range(8):
                    nc.vector.tensor_tensor(h_own[:, ct, :], x_sb[:, ct, :],
                                            rbc[:], OP.mult)

                # ===== pair exchange (AllGather, fp8) =====
                bounce = dram.tile([8, 128, 512], F8, tag="agin", name=f"agi{l}")
                for ct in range(8):
                    nc.sync.dma_start(bounce[ct], h_own[:, ct, :])
                gath = dram.tile([2, 8, 128, 512], F8, tag="agout",
                                 name=f"ago{l}")
                nc.gpsimd.collective_compute(
                    "AllGather", OP.bypass,
                    ins=[bounce[:].opt()],
                    outs=[gath[:].opt()],
                    replica_groups=RG,
                )

                sc_n1.__exit__(None, None, None)
                sc_q = nc.named_scope(f"q.{l}"); sc_q.__enter__()
                # Wv for this layer (no AG dependency -> overlaps exchange)
                wv_sb = wvp.tile([128, 2, 8, 512], F8, tag="wv", name=f"wv{l}")
                nc.sync.dma_start(wv_sb[:], wv_ext[l])

                # ===== q from h_own (overlaps AG) =====
                for hp in range(8):
                    wq_sb = wqkp.tile([128, 8, 128], F8, tag="wqk",
                                      name=f"wq{l}_{hp}")
                    nc.sync.dma_start(wq_sb[:], wq_ext[l, hp])
                    q_ps = ps512.tile([128, 512], F32, tag="p5", name=f"q{l}{hp}")
                    for cp in range(4):
                        nc.tensor.matmul(q_ps[:], wq_sb[:, 2 * cp:2 * cp + 2, :],
                                         h_own[:, 2 * cp:2 * cp + 2, :],
                                         start=(cp == 0), stop=(cp == 3),
                                         perf_mode=DR)
                    nc.vector.tensor_scalar_mul(q_sb[:, hp, :], q_ps[:],
                                                1.0 / SWQ)

                sc_q.__exit__(None, None, None)
                sc_kv = nc.named_scope(f"kv.{l}"); sc_kv.__enter__()
                # ===== scatter AG result into global-order strip =====
                # slot0 = half0 local blocks -> global [0,1,6,7]
                # slot1 = half1 local blocks -> global [2,3,4,5]
                for ct in range(8):
                    nc.sync.dma_start(h_str[:, ct, 0:256], gath[0, ct, :, 0:256])
                    nc.sync.dma_start(h_str[:, ct, 768:1024], gath[0, ct, :, 256:512])
                    nc.sync.dma_start(h_str[:, ct, 256:768], gath[1, ct])

                # ===== k over the strip =====
                for hp in range(8):
                    wk_sb = wqkp.tile([128, 8, 128], F8, tag="wqk",
                                      name=f"wk{l}_{hp}")
                    nc.sync.dma_start(wk_sb[:], wk_ext[l, hp])
                    for half in range(2):
                        k_pool, k_tag = ((ps512, "p5") if half == 0
                                         else (ps128, "pk"))
                        k_ps = k_pool.tile([128, 512], F32, tag=k_tag,
                                           name=f"k{l}{hp}{half}")
                        for cp in range(4):
                            nc.tensor.matmul(
                                k_ps[:], wk_sb[:, 2 * cp:2 * cp + 2, :],
                                h_str[:, 2 * cp:2 * cp + 2,
                                      half * 512:(half + 1) * 512],
                                start=(cp == 0), stop=(cp == 3), perf_mode=DR)
                        nc.vector.tensor_scalar_mul(
                            k_sb[:, hp, half * 512:(half + 1) * 512], k_ps[:],
                            1.0 / SW)

                sc_kv.__exit__(None, None, None)
                sc_at = nc.named_scope(f"at.{l}"); sc_at.__enter__()
                # ===== v tiles interleaved with per-head scores: the
                # scalar exp stream starts while the tensor engine is still
                # on v matmuls =====
                pending = []

                def _normalize(pend):
                    ph, pden = pend
                    php, poff = ph // 2, (ph % 2) * D
                    r = sp.tile([1, 512], BF16, tag="rex", name=f"re{l}_{ph}")
                    nc.scalar.activation(r[:], pden[:], AF.Exp, scale=-1.0)
                    rb_ps = ps128.tile([128, 512], F32, tag="pk",
                                       name=f"rb{l}_{ph}")
                    nc.tensor.matmul(rb_ps[poff:poff + D, :],
                                     ones_row_bf[:, 0:D],
                                     r[:], start=True, stop=True)
                    nc.vector.tensor_tensor(
                        o_f8[poff:poff + D, php, :], o_sb[poff:poff + D, php, :],
                        rb_ps[poff:poff + D, :], OP.mult)

                def v_tile(tkt):
                    for vh in range(2):
                        v_pool, v_tag = ((ps512, "p5") if vh == 0
                                         else (ps128, "pk"))
                        v_ps = v_pool.tile([128, 512], F32, tag=v_tag,
                                           name=f"v{l}{tkt}{vh}")
                        for cp in range(4):
                            nc.tensor.matmul(
                                v_ps[:],
                                h_str[:, 2 * cp:2 * cp + 2,
                                      tkt * 128:(tkt + 1) * 128],
                                wv_sb[:, vh, 2 * cp:2 * cp + 2, :],
                                start=(cp == 0), stop=(cp == 3), perf_mode=DR)
                        nc.vector.tensor_scalar_mul(
                            v_sb[:, tkt, 8 * vh:8 * vh + 8, 0:D],
                            v_ps[:].rearrange("p (q d) -> p q d", d=D),
                            1.0 / SW)
                    nc.gpsimd.memset(v_sb[:, tkt, :, D:66], 1.0)

                def head_scores(h16):
                    hp, off = h16 // 2, (h16 % 2) * D
                    ex = sp.tile([128, 8, 512], F8, tag="expA", bufs=3,
                                 name=f"ex{l}_{h16}")

                    def score_wide2(sp0):
                        # two wide k-tiles (s=sp0, sp0+1) into one 2-bank psum
                        # tile; a single [128,1024] exp drains both
                        w_ps = ps512.tile([128, 2, 512], F32, tag="p5",
                                          name=f"s{l}_{h16}_{sp0}")
                        for j in range(2):
                            s = sp0 + j
                            nc.tensor.matmul(
                                w_ps[:, j, :],
                                k_sb[off:off + D, hp, s * 128:(s + 1) * 128],
                                q_sb[off:off + D, hp, :],
                                start=True, stop=True)
                        nc.scalar.activation(ex[:, sp0:sp0 + 2, :], w_ps[:],
                                             AF.Exp)

                    def score_narrow(pi):
                        s_pool, s_tag = ((ps512, "p5") if pi == 0
                                         else (ps128, "pk"))
                        n_ps = s_pool.tile([128, 512], F32, tag=s_tag,
                                           name=f"n{l}_{h16}_{pi}")
                        for j in range(2):
                            s = 4 + 2 * pi + j
                            nc.tensor.matmul(
                                n_ps[:, 256 * j:256 * (j + 1)],
                                k_sb[off:off + D, hp, s * 128:(s + 1) * 128],
                                q_sb[off:off + D, hp, 256:512],
                                start=True, stop=True)
                        s0 = 4 + 2 * pi
                        nc.scalar.activation(
                            ex[:, s0:s0 + 2, 256:512],
                            n_ps[:].rearrange("p (s m) -> p s m", s=2),
                            AF.Exp)

                    def mask2(s0, c0):
                        nc.vector.tensor_tensor(
                            ex[:, s0:s0 + 2, c0:c0 + 256],
                            ex[:, s0:s0 + 2, c0:c0 + 256],
                            mask_sb[:, s0:s0 + 2, c0:c0 + 256], OP.mult)

                    score_wide2(0)
                    mask2(0, 0)
                    score_wide2(2)
                    mask2(2, 0)
                    score_narrow(0)
                    mask2(4, 256)
                    score_narrow(1)
                    mask2(6, 256)
                    return ex

                def head_avs(h16, ex):
                    hp, off = h16 // 2, (h16 % 2) * D
                    o_ps = ps128o.tile([128, 512], F32, tag="po",
                                       name=f"o{l}_{h16}")

                    def av_pair(p):
                        s2 = 2 * p
                        vv = v_sb[:, s2:s2 + 2, h16, 0:OUT]
                        if p == 0:
                            nc.tensor.matmul(o_ps[:OUT, :], vv,
                                             ex[:, s2:s2 + 2, :],
                                             start=True, stop=False,
                                             perf_mode=DR)
                        elif p == 1:
                            nc.tensor.matmul(o_ps[:OUT, 0:256], vv,
                                             ex[:, s2:s2 + 2, 0:256],
                                             start=False, stop=True,
                                             perf_mode=DR)
                            nc.tensor.matmul(o_ps[:OUT, 256:512], vv,
                                             ex[:, s2:s2 + 2, 256:512],
                                             start=False, stop=False,
                                             perf_mode=DR)
                        else:
                            nc.tensor.matmul(o_ps[:OUT, 256:512], vv,
                                             ex[:, s2:s2 + 2, 256:512],
                                             start=False, stop=(p == 3),
                                             perf_mode=DR)

                    for p in range(4):
                        av_pair(p)
                    den = sp.tile([1, 512], F32, tag="rr", name=f"r{l}_{h16}")
                    nc.scalar.activation(den[:], o_ps[VOCAB:OUT, :], AF.Ln)
                    nc.vector.tensor_copy(o_sb[off:off + D, hp, :], o_ps[:D, :])
                    pending.append((h16, den))
                    while len(pending) > 1:
                        _normalize(pending.pop(0))

                for tkt in range(8):
                    v_tile(tkt)
                exs = {0: head_scores(0)}
                for h16 in range(16):
                    if h16 + 1 < 16:
                        exs[h16 + 1] = head_scores(h16 + 1)
                    head_avs(h16, exs.pop(h16))
                while pending:
                    _normalize(pending.pop(0))

                sc_at.__exit__(None, None, None)
                sc_wo = nc.named_scope(f"wo.{l}"); sc_wo.__enter__()
                # ===== Wo + residual =====
                for cot in range(8):
                    wo_sb = w512p.tile([128, 8, 128], F8, tag="w5",
                                       name=f"wo{l}_{cot}")
                    nc.sync.dma_start(wo_sb[:], wo_ext[l, cot])
                    xo_ps = ps512.tile([128, 512], F32, tag="p5",
                                       name=f"xo{l}{cot}")
                    for hdp in range(4):
                        nc.tensor.matmul(xo_ps[:],
                                         wo_sb[:, 2 * hdp:2 * hdp + 2, :],
                                         o_f8[:, 2 * hdp:2 * hdp + 2, :],
                                         start=(hdp == 0), stop=(hdp == 3),
                                         perf_mode=DR)
                    xo_sb = sp.tile([128, 512], F32, tag="xo", name=f"xs{l}{cot}")
                    nc.scalar.activation(xo_sb[:], xo_ps[:], AF.Identity,
                                         bias=bo_sb[:, l, cot, :],
                                         scale=1.0 / SW)
                    nc.vector.tensor_add(x_sb[:, cot, :], x_sb[:, cot, :],
                                         xo_sb[:])

                sc_wo.__exit__(None, None, None)
                sc_n2 = nc.named_scope(f"n2.{l}"); sc_n2.__enter__()
                # ===== norm2 -> h2 (fp8) =====
                h2_sb = bigp.tile([128, 8, 512], F8, tag="big", name=f"h2_{l}")
                rbc2 = rms_rbc(f"b{l}")
                for ct in range(8):
                    nc.vector.tensor_tensor(h2_sb[:, ct, :], x_sb[:, ct, :],
                                            rbc2[:], OP.mult)

                sc_n2.__exit__(None, None, None)
                sc_ff = nc.named_scope(f"ff.{l}"); sc_ff.__enter__()
                # ===== FFN (ft chunks of 4; fp8 DoubleRow both matmuls) =====
                def emit_w2(chunk, u_prev, w2c):
                    for cot in range(8):
                        y_ps = ps128o.tile([128, 512], F32, tag="po",
                                          name=f"y{l}{chunk}{cot}")
                        for p in range(2):
                            nc.tensor.matmul(y_ps[:], w2c[p][:, :, cot, :],
                                             u_prev[:, 2 * p:2 * p + 2, :],
                                             start=(p == 0), stop=(p == 1),
                                             perf_mode=DR)
                        nc.vector.scalar_tensor_tensor(
                            x_sb[:, cot, :], y_ps[:], 1.0 / SW,
                            x_sb[:, cot, :], OP.mult, OP.add)
                        if chunk == 0:
                            # b2 folded in early: runs on scalar while later
                            # chunks' matmuls stream, off the layer-end path
                            nc.scalar.add(x_sb[:, cot, :], x_sb[:, cot, :],
                                          b2_sb[:, l, cot, :])

                prev = None
                for chunk in range(8):
                    u_sb = sp.tile([128, 4, 512], F8, tag="u",
                                   name=f"u{l}_{chunk}")
                    w2c = []
                    for fi in range(4):
                        ft = chunk * 4 + fi
                        w1_sb = w512p.tile([128, 8, 128], F8, tag="w5",
                                           name=f"w1_{l}_{ft}")
                        nc.sync.dma_start(w1_sb[:], w1_ext[l, ft])
                        if fi % 2 == 0:
                            w2_sb = w2p.tile([128, 2, 8, 128], F8, tag="w2",
                                             name=f"w2_{l}_{chunk}_{fi // 2}")
                            nc.sync.dma_start(w2_sb[:],
                                              w2_ext[l, chunk * 2 + fi // 2])
                            w2c.append(w2_sb)
                        u_pool, u_tag = ((ps512, "p5") if fi % 2 == 0
                                         else (ps128, "pk"))
                        u_ps = u_pool.tile([128, 512], F32, tag=u_tag,
                                           name=f"u{l}{ft}")
                        for cp in range(4):
                            nc.tensor.matmul(u_ps[:],
                                             w1_sb[:, 2 * cp:2 * cp + 2, :],
                                             h2_sb[:, 2 * cp:2 * cp + 2, :],
                                             start=(cp == 0), stop=(cp == 3),
                                             perf_mode=DR)
                        nc.scalar.activation(u_sb[:, fi, :], u_ps[:], AF.Gelu,
                                             bias=b1_sb[:, l, ft, :],
                                             scale=1.0 / SW)
                    if prev is not None:
                        emit_w2(*prev)
                    prev = (chunk, u_sb, w2c)
                emit_w2(*prev)


                if debug and l == 0:
                    nc.sync.dma_start(dbg_h[:], h_str[:])
                    nc.sync.dma_start(dbg_k[:], k_sb[:])
                    nc.sync.dma_start(dbg_q[:], q_sb[:])
                    nc.sync.dma_start(dbg_v[:], v_sb[:])
                    nc.sync.dma_start(dbg_o[:], o_f8[:])
                    nc.sync.dma_start(dbg_u[:], u_sb[:])
                    nc.sync.dma_start(dbg_x2[:], x_sb[:])

                sc_ff.__exit__(None, None, None)

            # ===== lm head + log_softmax / log_sigmoid =====
            for tlt in range(4):
                lg = ps512.tile([128, OUT], F32, tag="p5", name=f"lg{tlt}")
                for ct in range(8):
                    xr = sp.tile([128, 128], BF16, tag="xr", name=f"xr{tlt}_{ct}")
                    nc.scalar.copy(xr[:], x_sb[:, ct, tlt * 128:(tlt + 1) * 128])
                    nc.tensor.matmul(lg[:], xr[:], lmw_sb[:, ct, :],
                                     start=(ct == 0), stop=(ct == 7))
                lgb = sp.tile([128, OUT], F32, tag="lgb", name=f"lgb{tlt}")
                nc.vector.tensor_add(lgb[:], lg[:], lmb_sb[:])
                m = sp.tile([128, 1], F32, tag="m", name=f"m{tlt}")
                nc.vector.reduce_max(m[:], lgb[:, 0:VOCAB], axis=AX.X)
                nm = sp.tile([128, 1], F32, tag="nm", name=f"nm{tlt}")
                nc.scalar.mul(nm[:], m[:], -1.0)
                e = sp.tile([128, VOCAB], F32, tag="e", name=f"e{tlt}")
                es = sp.tile([128, 1], F32, tag="es", name=f"es{tlt}")
                nc.scalar.activation(e[:], lgb[:, 0:VOCAB], AF.Exp, bias=nm[:],
                                     accum_out=es[:])
                lse = sp.tile([128, 1], F32, tag="lse", name=f"lse{tlt}")
                nc.scalar.activation(lse[:], es[:], AF.Ln)
                bt = sp.tile([128, 1], F32, tag="bt", name=f"bt{tlt}")
                nc.vector.tensor_tensor(bt[:], nm[:], lse[:], OP.subtract)
                outt = sp.tile([128, OUT], F32, tag="outt", name=f"ot{tlt}")
                nc.scalar.activation(outt[:, 0:VOCAB], lgb[:, 0:VOCAB],
                                     AF.Identity, bias=bt[:])
                sg = sp.tile([128, 1], F32, tag="sg", name=f"sg{tlt}")
                nc.scalar.activation(sg[:], lgb[:, VOCAB:OUT], AF.Sigmoid)
                nc.scalar.activation(outt[:, VOCAB:OUT], sg[:], AF.Ln)
                nc.sync.dma_start(out_ext[tlt * 128:(tlt + 1) * 128, :], outt[:])

    _split_sync_waits(nc)
    return nc


# ---------------------------------------------------------------------------
# host-side preparation
# ---------------------------------------------------------------------------
def _own_rows(core):
    return np.concatenate(
        [np.arange(b * 128, (b + 1) * 128) for b in OWN_BLOCKS[core % 2]]
    )


def _bf(a):
    return np.asarray(a, dtype=ml_dtypes.bfloat16)


def _f8(a, s):
    return np.clip(np.asarray(a, np.float32) * s, -240.0, 240.0).astype(
        ml_dtypes.float8_e4m3fn
    )


def _f32(a):
    return np.ascontiguousarray(a, dtype=np.float32)


def _prep(inputs):
    acts = np.asarray(inputs["acts"])
    durations = _f32(inputs["durations"])
    emb_table = _f32(inputs["emb_table"])
    pos_table = _f32(inputs["pos_table"])
    Wq, Wk, Wv = (_f32(inputs[k]) for k in ("Wq", "Wk", "Wv"))
    Wo, bo = _f32(inputs["Wo"]), _f32(inputs["bo"])
    W1, b1 = _f32(inputs["W1"]), _f32(inputs["b1"])
    W2, b2 = _f32(inputs["W2"]), _f32(inputs["b2"])
    g1, g2 = _f32(inputs["g1"]), _f32(inputs["g2"])
    lm_W, lm_b = _f32(inputs["lm_W"]), _f32(inputs["lm_b"])

    # fold g1 into Wq/Wk/Wv (q also gets the D^-0.5 score scale), g2 into W1
    Wq_eff = Wq * g1[:, None, :, None] * (D ** -0.5)
    Wk_eff = Wk * g1[:, None, :, None]
    Wv_eff = Wv * g1[:, None, :, None]
    W1_eff = W1 * g2[:, :, None]

    def qk_arr(A, s):  # [L,H,C,D] -> [L, hp, cp, ct, m], fp8 scaled by s
        A2 = A.transpose(0, 2, 1, 3).reshape(L, C, H * D)
        return _f8(A2.reshape(L, 8, 128, 8, 128).transpose(0, 3, 2, 1, 4), s)

    shared = {
        "aug_table": None, "ones_col": _bf(np.ones((128, 1))),
        "ones_row": _f32(np.ones((1, 128))),
        "ones_row_bf": _bf(np.ones((1, 128))),
        "Wq_arr": qk_arr(Wq_eff, SWQ), "Wk_arr": qk_arr(Wk_eff, SW),
        "Wv_arr": _f8(Wv_eff.transpose(0, 2, 1, 3).reshape(L, C, H * D)
                      .reshape(L, 8, 128, 2, 512).transpose(0, 2, 3, 1, 4), SW),
        "Wo_arr": _f8(Wo.reshape(L, 8, 128, 8, 128).transpose(0, 3, 2, 1, 4), SW),
        "W1_arr": _f8(W1_eff.reshape(L, 8, 128, 32, 128)
                      .transpose(0, 3, 2, 1, 4), SW),
        "W2_arr": _f8(W2.reshape(L, 16, 2, 128, 8, 128)
                      .transpose(0, 1, 3, 2, 4, 5), SW),
        "bo_fm": np.ascontiguousarray(
            bo.reshape(L, 8, 128).transpose(2, 0, 1))[..., None],
        "b1_fm": np.ascontiguousarray(
            b1.reshape(L, 32, 128).transpose(2, 0, 1))[..., None],
        "b2_fm": np.ascontiguousarray(
            b2.reshape(L, 8, 128).transpose(2, 0, 1))[..., None],
        "lmW_arr": _bf(lm_W.reshape(8, 128, OUT).transpose(1, 0, 2)),
        "lmb_bc": _f32(np.tile(lm_b[None, :], (128, 1))),
    }
    aug = np.zeros((OUT, C), np.float32)
    aug[:VOCAB, : C - 1] = emb_table
    aug[VOCAB, C - 1] = 1.0
    shared["aug_table"] = _f32(aug.reshape(OUT, 8, 128))

    in_maps = []
    for core in range(8):
        b, half = core // 2, core % 2
        rows = _own_rows(core)
        oh = np.zeros((OUT, 512), np.float32)
        oh[acts[b, rows], np.arange(512)] = 1.0
        oh[VOCAB, :] = durations[b, rows]
        pos = pos_table[rows].T.reshape(8, 128, 512).transpose(1, 0, 2)
        masks = np.ones((8, 128, 512), np.float32)
        for s in range(8):
            gk = s * 128
            ii = gk + np.arange(128)[:, None]
            for j in range(4):
                gq = OWN_BLOCKS[half][j] * 128
                jj = gq + np.arange(128)[None, :]
                masks[s, :, j * 128:(j + 1) * 128] = (ii <= jj)
        m = dict(shared)
        m["onehot_t"] = _f32(oh)
        m["pos_fm"] = _bf(pos)
        m["masks"] = _bf(masks.transpose(1, 0, 2))
        in_maps.append(m)
    return in_maps


LAST_EXEC_NS = [None]
LAST_SCOPES = [None]


def kernel(**inputs) -> np.ndarray:
    nc = build_graph()
    in_maps = _prep(inputs)
    trace = bool(int(os.environ.get("KERNEL_TRACE", "0")))
    res = bass_utils.run_bass_kernel_spmd(
        nc, in_maps, list(range(8)), trace=trace,
        trace_cores=[0] if trace else None,
    )
    LAST_EXEC_NS[0] = res.exec_time_ns
    LAST_SCOPES[0] = res.per_core_scope_times
    if trace and res.instructions_and_trace:
        print("trace path:", res.instructions_and_trace[1])
    full = np.zeros((B, T, OUT), np.float32)
    for core in range(8):
        full[core // 2, _own_rows(core)] = res.results[core]["out"]
    return full



# revision 44
# speedup vs baseline: 1.0259x; 1.0259x over previous
"""Trainium2 Bass kernel for nn_AttentionDecoder_82738249990894 (B=4, T=1024,
C=1024, H=16, D=64, F=4096, L=4, vocab 64+1 outputs).

Sharding: sequence-split data parallel over 8 cores.  Core c handles batch
b = c//2, sequence half = c%2.  Balanced causal split: half0 owns global
128-row blocks [0,1,6,7], half1 owns [2,3,4,5] (equal attention work: both
see 18 causal k-tiles).  Per layer the pair exchanges rmsnorm'd activations
(bf16, pairwise AllGather, ~1MB) and each core recomputes k/v for all 1024
tokens locally.  No other communication.

SPMD uniformity: one graph runs on all 8 cores, so the key/value strip is
kept in GLOBAL token order (the AllGather return scatters both pair slots
to fixed global positions) and every local q-tile j computes scores against
the union visibility vis_u=[3,4,7,8] k-tiles; per-core 0/1 masks (input
data) encode causality and half-dependent visibility.

Matmul dtypes: fp8e4m3 with DoubleRow perf mode (2 k-tiles per pass, 2x
PE throughput) for the qkv/Wo/W1/W2/AV matmuls; weights are pre-scaled
(x64, x512 for Wq) into fp8's dynamic range on the host and the inverse
scale is folded into the psum->sbuf copy or activation that follows.
Scores stay bf16 (64-deep contraction, DoubleRow inapplicable).
Residual x stays fp32.  Softmax skips max-subtraction (scores are O(1);
fp32 psum exp is safe) and gets denominators free via a ones-column
appended to v; normalization is deferred to after the AV matmul.
"""
import os
import sys
import types

sys.path.insert(0, "/opt/trn_rl_repo")

import numpy as np
import ml_dtypes

import antenv

if not hasattr(antenv, "axon_hooks"):
    _mod = types.ModuleType("antenv.axon_hooks")
    _mod._hook = None
    _mod.set_axon_ntff_profile_hook = lambda h: setattr(_mod, "_hook", h)
    _mod.get_axon_ntff_profile_hook = lambda: _mod._hook
    sys.modules["antenv.axon_hooks"] = _mod
    antenv.axon_hooks = _mod
    try:
        from trn_agent_boot.trn_boot import _ntff_profile_via_ctypes

        _mod.set_axon_ntff_profile_hook(
            _ntff_profile_via_ctypes("/opt/axon/libaxon_pjrt.so")
        )
    except Exception:
        pass

import concourse.bass as bass
import concourse.mybir as mybir
import concourse.tile as tile
from concourse import bass_utils

bass_utils.upload_artifacts = lambda tmpdir: "local://" + tmpdir
try:
    from concourse import tile_utils as _tu

    _tu.max_sbuf_usage = 206 * 1024
except Exception:
    pass

F32 = mybir.dt.float32
F32R = mybir.dt.float32r
BF16 = mybir.dt.bfloat16
F8 = mybir.dt.float8e4
DR = mybir.MatmulPerfMode.DoubleRow
AF = mybir.ActivationFunctionType
OP = mybir.AluOpType
AX = mybir.AxisListType

SW = 64.0     # fp8 weight pre-scale (Wk/Wv/Wo/W1/W2)
SWQ = 512.0   # Wq pre-scale (D^-0.5 folded in makes it 8x smaller)

B, T, C, H, D, F, L = 4, 1024, 1024, 16, 64, 4096, 4
VOCAB, OUT = 64, 65
EPS = float(np.finfo(np.float32).eps)
RG = [[0, 1], [2, 3], [4, 5], [6, 7]]
OWN_BLOCKS = {0: [0, 1, 6, 7], 1: [2, 3, 4, 5]}
VIS_U = [3, 4, 7, 8]          # union visible k-tiles per local q-tile
N_MASK = 3                    # last 3 visible slots carry a mask

_wsplit_ctr = [0]


def _split_sync_waits(nc):
    """This walrus build allows one sync-wait per instruction; hoist extras
    onto injected same-engine NoOps."""
    for f in nc.m.functions:
        for bb in f.blocks:
            out = []
            changed = False
            for inst in bb.instructions:
                si = getattr(inst, "sync_info", None)
                if si is not None and si.on_wait is not None and len(si.on_wait) > 1:
                    waits = list(si.on_wait)
                    for w in waits[:-1]:
                        _wsplit_ctr[0] += 1
                        n = mybir.InstNoOp(
                            name=f"WSPLIT-{_wsplit_ctr[0]}", ins=[], outs=[]
                        )
                        n.engine = inst.engine
                        n.sync_info = mybir.SyncInfo(on_wait=[w], on_update=[])
                        out.append(n)
                    inst.sync_info = mybir.SyncInfo(
                        on_wait=[waits[-1]], on_update=list(si.on_update)
                    )
                    changed = True
                out.append(inst)
            if changed:
                bb.instructions[:] = out


def build_graph():
    nc = bass.Bass()
    dp = nc.declare_dram_parameter
    onehot_ext = dp("onehot_t", [OUT, 512], F32R, isOutput=False)
    pos_ext = dp("pos_fm", [128, 8, 512], BF16, isOutput=False)
    aug_ext = dp("aug_table", [OUT, 8, 128], F32R, isOutput=False)
    mask_ext = dp("masks", [128, 8, 512], BF16, isOutput=False)
    onescol_ext = dp("ones_col", [128, 1], BF16, isOutput=False)
    onesrow_ext = dp("ones_row", [1, 128], F32R, isOutput=False)
    onesrowb_ext = dp("ones_row_bf", [1, 128], BF16, isOutput=False)
    wq_ext = dp("Wq_arr", [L, 8, 128, 8, 128], F8, isOutput=False)
    wk_ext = dp("Wk_arr", [L, 8, 128, 8, 128], F8, isOutput=False)
    wv_ext = dp("Wv_arr", [L, 128, 2, 8, 512], F8, isOutput=False)
    wo_ext = dp("Wo_arr", [L, 8, 128, 8, 128], F8, isOutput=False)
    w1_ext = dp("W1_arr", [L, 32, 128, 8, 128], F8, isOutput=False)
    w2_ext = dp("W2_arr", [L, 16, 128, 2, 8, 128], F8, isOutput=False)
    bo_ext = dp("bo_fm", [128, L, 8, 1], F32, isOutput=False)
    b1_ext = dp("b1_fm", [128, L, 32, 1], F32, isOutput=False)
    b2_ext = dp("b2_fm", [128, L, 8, 1], F32, isOutput=False)
    lmw_ext = dp("lmW_arr", [128, 8, OUT], BF16, isOutput=False)
    lmb_ext = dp("lmb_bc", [128, OUT], F32, isOutput=False)
    out_ext = dp("out", [512, OUT], F32, isOutput=True)
    debug = bool(int(os.environ.get("KERNEL_DEBUG", "0")))
    if debug:
        dbg_h = dp("dbg_h", [128, 8, 1024], F8, isOutput=True)
        dbg_k = dp("dbg_k", [128, 8, 1024], BF16, isOutput=True)
        dbg_q = dp("dbg_q", [128, 8, 512], BF16, isOutput=True)
        dbg_v = dp("dbg_v", [128, 8, 16, 66], F8, isOutput=True)
        dbg_o = dp("dbg_o", [128, 8, 512], F8, isOutput=True)
        dbg_u = dp("dbg_u", [128, 4, 512], F8, isOutput=True)
        dbg_x2 = dp("dbg_x2", [128, 8, 512], F32, isOutput=True)

    with tile.TileContext(nc) as tc:
        nc_lp = nc.allow_low_precision(reason="bf16 attention path is intentional")
        nc_lp.__enter__()
        with (
            tc.tile_pool(name="persist", bufs=1) as pp,
            tc.tile_pool(name="scratch", bufs=2) as sp,
            tc.tile_pool(name="wqk", bufs=3) as wqkp,
            tc.tile_pool(name="w512", bufs=3) as w512p,
            tc.tile_pool(name="w2p", bufs=5) as w2p,
            tc.tile_pool(name="bigp", bufs=1) as bigp,
            tc.tile_pool(name="wvp", bufs=1) as wvp,
            tc.tile_pool(name="ps512", bufs=3, space="PSUM") as ps512,
            tc.tile_pool(name="ps128", bufs=2, space="PSUM") as ps128,
            tc.tile_pool(name="ps128o", bufs=3, space="PSUM") as ps128o,
            tc.tile_pool(name="dram", bufs=2, space="DRAM") as dram,
        ):
            # ---- constants ----
            ones_col = pp.tile([128, 1], BF16)
            ones_row = pp.tile([1, 128], F32R)
            ones_row_bf = pp.tile([1, 128], BF16)
            aug_sb = pp.tile([OUT, 8, 128], F32R)
            onehot_sb = pp.tile([OUT, 512], F32R)
            mask_sb = pp.tile([128, 8, 512], BF16)
            lmw_sb = pp.tile([128, 8, OUT], BF16)
            lmb_sb = pp.tile([128, OUT], F32)
            bo_sb = pp.tile([128, L, 8, 1], F32)
            b1_sb = pp.tile([128, L, 32, 1], F32)
            b2_sb = pp.tile([128, L, 8, 1], F32)
            nc.sync.dma_start(aug_sb[:], aug_ext[:])
            nc.sync.dma_start(onehot_sb[:], onehot_ext[:])
            nc.sync.dma_start(ones_col[:], onescol_ext[:])
            nc.sync.dma_start(ones_row[:], onesrow_ext[:])
            nc.sync.dma_start(ones_row_bf[:], onesrowb_ext[:])

            # tiny warm-up collective: absorbs first-use CC latency while
            # the tensor engine runs the embedding
            cw_in = dram.tile([1, 64], F32, tag="cwi", name="cwi")
            cw_out = dram.tile([2, 64], F32, tag="cwo", name="cwo")
            cw_sb = pp.tile([1, 64], F32)
            nc.gpsimd.memset(cw_sb[:], 0.0)
            nc.sync.dma_start(cw_in[:], cw_sb[:])
            nc.gpsimd.collective_compute(
                "AllGather", OP.bypass,
                ins=[cw_in[:].opt()],
                outs=[cw_out[:].opt()],
                replica_groups=RG,
            )

            nc.sync.dma_start(mask_sb[:], mask_ext[:])
            nc.sync.dma_start(lmw_sb[:], lmw_ext[:])
            nc.sync.dma_start(lmb_sb[:], lmb_ext[:])
            nc.sync.dma_start(bo_sb[:], bo_ext[:])
            nc.sync.dma_start(b1_sb[:], b1_ext[:])
            nc.sync.dma_start(b2_sb[:], b2_ext[:])

            eps_sb = pp.tile([128, 1], F32)
            nc.gpsimd.memset(eps_sb[:], EPS)

            # ---- persistent activations ----
            x_sb = pp.tile([128, 8, 512], F32)       # residual (feature-major)
            h_own = pp.tile([128, 8, 512], F8)       # norm'd own tokens
            h_str = pp.tile([128, 8, 1024], F8)      # norm'd pair, global order
            q_sb = pp.tile([128, 8, 512], BF16)      # [2h*64, hp, local t]
            k_sb = pp.tile([128, 8, 1024], BF16)     # [2h*64, hp, global t]
            # inner dim padded 65->66 so every fp8 slice lands on an even
            # byte offset (odd SBUF offsets break engine reads)
            v_sb = pp.tile([128, 8, 16, 66], F8)     # [tk, tkt, head, d+1+pad]
            o_sb = pp.tile([128, 8, 512], BF16)      # attn out, unnormalized
            o_f8 = pp.tile([128, 8, 512], F8)        # normalized attn out
            # pos and per-layer h2 share one big slot (disjoint lifetimes)
            pos_sb = bigp.tile([128, 8, 512], BF16, tag="big", name="pos")
            nc.sync.dma_start(pos_sb[:], pos_ext[:])

            # ---- embedding: x = onehot @ aug_table + pos ----
            sc_emb = nc.named_scope("emb"); sc_emb.__enter__()
            for ct in range(8):
                emb_ps = ps512.tile([128, 512], F32, tag="p5", name=f"emb{ct}")
                nc.tensor.matmul(emb_ps[:], aug_sb[:, ct, :], onehot_sb[:],
                                 start=True, stop=True)
                nc.vector.tensor_add(x_sb[:, ct, :], emb_ps[:], pos_sb[:, ct, :])

            sc_emb.__exit__(None, None, None)

            def rms_rbc(tag):
                ssum = ps512.tile([128, 512], F32, tag="p5", name=f"ss{tag}")
                for ct in range(8):
                    xsq = sp.tile([128, 512], BF16, tag="xsq", name=f"xq{tag}{ct}")
                    if ct % 2 == 0:
                        nc.scalar.activation(xsq[:], x_sb[:, ct, :], AF.Square)
                    else:
                        nc.vector.tensor_tensor(xsq[:], x_sb[:, ct, :],
                                                x_sb[:, ct, :], OP.mult)
                    nc.tensor.matmul(ssum[:1, :], ones_col[:], xsq[:],
                                     start=(ct == 0), stop=(ct == 7))
                lnv = sp.tile([1, 512], F32, tag="lnv", name=f"lv{tag}", bufs=1)
                nc.scalar.activation(lnv[:], ssum[:1, :], AF.Ln,
                                     bias=eps_sb[:1, :], scale=1.0 / C)
                rstd = sp.tile([1, 512], F32R, tag="sqv", name=f"sv{tag}",
                               bufs=1)
                nc.scalar.activation(rstd[:], lnv[:], AF.Exp, scale=-0.5)
                rbc = ps512.tile([128, 512], F32, tag="p5", name=f"rb{tag}")
                nc.tensor.matmul(rbc[:], ones_row[:], rstd[:], start=True,
                                 stop=True)
                return rbc

            for l in range(L):
                sc_n1 = nc.named_scope(f"n1.{l}"); sc_n1.__enter__()
                # ===== norm1 -> h_own =====
                rbc = rms_rbc(f"a{l}")
                for ct in range(8):
                    nc.vector.tensor_tensor(h_own[:, ct, :], x_sb[:, ct, :],
                                            rbc[:], OP.mult)

                # ===== pair exchange (AllGather, fp8) =====
                bounce = dram.tile([8, 128, 512], F8, tag="agin", name=f"agi{l}")
                for ct in range(8):
                    nc.sync.dma_start(bounce[ct], h_own[:, ct, :])
                gath = dram.tile([2, 8, 128, 512], F8, tag="agout",
                                 name=f"ago{l}")
                nc.gpsimd.collective_compute(
                    "AllGather", OP.bypass,
                    ins=[bounce[:].opt()],
                    outs=[gath[:].opt()],
                    replica_groups=RG,
                )

                sc_n1.__exit__(None, None, None)
                sc_q = nc.named_scope(f"q.{l}"); sc_q.__enter__()
                # Wv for this layer (no AG dependency -> overlaps exchange)
                wv_sb = wvp.tile([128, 2, 8, 512], F8, tag="wv", name=f"wv{l}")
                nc.sync.dma_start(wv_sb[:], wv_ext[l])

                # ===== q from h_own (overlaps AG) =====
                for hp in range(8):
                    wq_sb = wqkp.tile([128, 8, 128], F8, tag="wqk",
                                      name=f"wq{l}_{hp}")
                    nc.sync.dma_start(wq_sb[:], wq_ext[l, hp])
                    q_ps = ps512.tile([128, 512], F32, tag="p5", name=f"q{l}{hp}")
                    for cp in range(4):
                        nc.tensor.matmul(q_ps[:], wq_sb[:, 2 * cp:2 * cp + 2, :],
                                         h_own[:, 2 * cp:2 * cp + 2, :],
                                         start=(cp == 0), stop=(cp == 3),
                                         perf_mode=DR)
                    nc.vector.tensor_scalar_mul(q_sb[:, hp, :], q_ps[:],
                                                1.0 / SWQ)

                sc_q.__exit__(None, None, None)
                sc_kv = nc.named_scope(f"kv.{l}"); sc_kv.__enter__()
                # ===== scatter AG result into global-order strip =====
                # slot0 = half0 local blocks -> global [0,1,6,7]
                # slot1 = half1 local blocks -> global [2,3,4,5]
                for ct in range(8):
                    nc.sync.dma_start(h_str[:, ct, 0:256], gath[0, ct, :, 0:256])
                    nc.sync.dma_start(h_str[:, ct, 768:1024], gath[0, ct, :, 256:512])
                    nc.sync.dma_start(h_str[:, ct, 256:768], gath[1, ct])

                # ===== k over the strip =====
                for hp in range(8):
                    wk_sb = wqkp.tile([128, 8, 128], F8, tag="wqk",
                                      name=f"wk{l}_{hp}")
                    nc.sync.dma_start(wk_sb[:], wk_ext[l, hp])
                    for half in range(2):
                        k_pool, k_tag = ((ps512, "p5") if half == 0
                                         else (ps128, "pk"))
                        k_ps = k_pool.tile([128, 512], F32, tag=k_tag,
                                           name=f"k{l}{hp}{half}")
                        for cp in range(4):
                            nc.tensor.matmul(
                                k_ps[:], wk_sb[:, 2 * cp:2 * cp + 2, :],
                                h_str[:, 2 * cp:2 * cp + 2,
                                      half * 512:(half + 1) * 512],
                                start=(cp == 0), stop=(cp == 3), perf_mode=DR)
                        nc.vector.tensor_scalar_mul(
                            k_sb[:, hp, half * 512:(half + 1) * 512], k_ps[:],
                            1.0 / SW)

                sc_kv.__exit__(None, None, None)
                sc_at = nc.named_scope(f"at.{l}"); sc_at.__enter__()
                # ===== v tiles interleaved with per-head scores: the
                # scalar exp stream starts while the tensor engine is still
                # on v matmuls =====
                pending = []

                def _normalize(pend):
                    ph, pden = pend
                    php, poff = ph // 2, (ph % 2) * D
                    r = sp.tile([1, 512], BF16, tag="rex", name=f"re{l}_{ph}")
                    nc.scalar.activation(r[:], pden[:], AF.Exp, scale=-1.0)
                    rb_ps = ps128.tile([128, 512], F32, tag="pk",
                                       name=f"rb{l}_{ph}")
                    nc.tensor.matmul(rb_ps[poff:poff + D, :],
                                     ones_row_bf[:, 0:D],
                                     r[:], start=True, stop=True)
                    nc.vector.tensor_tensor(
                        o_f8[poff:poff + D, php, :], o_sb[poff:poff + D, php, :],
                        rb_ps[poff:poff + D, :], OP.mult)

                def v_tile(tkt):
                    for vh in range(2):
                        v_pool, v_tag = ((ps512, "p5") if vh == 0
                                         else (ps128, "pk"))
                        v_ps = v_pool.tile([128, 512], F32, tag=v_tag,
                                           name=f"v{l}{tkt}{vh}")
                        for cp in range(4):
                            nc.tensor.matmul(
                                v_ps[:],
                                h_str[:, 2 * cp:2 * cp + 2,
                                      tkt * 128:(tkt + 1) * 128],
                                wv_sb[:, vh, 2 * cp:2 * cp + 2, :],
                                start=(cp == 0), stop=(cp == 3), perf_mode=DR)
                        nc.vector.tensor_scalar_mul(
                            v_sb[:, tkt, 8 * vh:8 * vh + 8, 0:D],
                            v_ps[:].rearrange("p (q d) -> p q d", d=D),
                            1.0 / SW)
                    nc.gpsimd.memset(v_sb[:, tkt, :, D:66], 1.0)

                def head_scores(h16):
                    hp, off = h16 // 2, (h16 % 2) * D
                    ex = sp.tile([128, 8, 512], F8, tag="expA", bufs=3,
                                 name=f"ex{l}_{h16}")

                    def score_wide(s):
                        s_pool, s_tag = ((ps512, "p5") if s % 2 == 0
                                         else (ps128, "pk"))
                        s_ps = s_pool.tile([128, 512], F32, tag=s_tag,
                                           name=f"s{l}_{h16}_{s}")
                        nc.tensor.matmul(
                            s_ps[:],
                            k_sb[off:off + D, hp, s * 128:(s + 1) * 128],
                            q_sb[off:off + D, hp, :],
                            start=True, stop=True)
                        nc.scalar.activation(ex[:, s, :], s_ps[:], AF.Exp)

                    def score_narrow(pi):
                        s_pool, s_tag = ((ps512, "p5") if pi == 0
                                         else (ps128, "pk"))
                        n_ps = s_pool.tile([128, 512], F32, tag=s_tag,
                                           name=f"n{l}_{h16}_{pi}")
                        for j in range(2):
                            s = 4 + 2 * pi + j
                            nc.tensor.matmul(
                                n_ps[:, 256 * j:256 * (j + 1)],
                                k_sb[off:off + D, hp, s * 128:(s + 1) * 128],
                                q_sb[off:off + D, hp, 256:512],
                                start=True, stop=True)
                        s0 = 4 + 2 * pi
                        nc.scalar.activation(
                            ex[:, s0:s0 + 2, 256:512],
                            n_ps[:].rearrange("p (s m) -> p s m", s=2),
                            AF.Exp)

                    def mask2(s0, c0):
                        nc.vector.tensor_tensor(
                            ex[:, s0:s0 + 2, c0:c0 + 256],
                            ex[:, s0:s0 + 2, c0:c0 + 256],
                            mask_sb[:, s0:s0 + 2, c0:c0 + 256], OP.mult)

                    score_wide(0)
                    score_wide(1)
                    mask2(0, 0)
                    score_wide(2)
                    score_wide(3)
                    mask2(2, 0)
                    score_narrow(0)
                    mask2(4, 256)
                    score_narrow(1)
                    mask2(6, 256)
                    return ex

                def head_avs(h16, ex):
                    hp, off = h16 // 2, (h16 % 2) * D
                    o_ps = ps128o.tile([128, 512], F32, tag="po",
                                       name=f"o{l}_{h16}")

                    def av_pair(p):
                        s2 = 2 * p
                        vv = v_sb[:, s2:s2 + 2, h16, 0:OUT]
                        if p == 0:
                            nc.tensor.matmul(o_ps[:OUT, :], vv,
                                             ex[:, s2:s2 + 2, :],
                                             start=True, stop=False,
                                             perf_mode=DR)
                        elif p == 1:
                            nc.tensor.matmul(o_ps[:OUT, 0:256], vv,
                                             ex[:, s2:s2 + 2, 0:256],
                                             start=False, stop=True,
                                             perf_mode=DR)
                            nc.tensor.matmul(o_ps[:OUT, 256:512], vv,
                                             ex[:, s2:s2 + 2, 256:512],
                                             start=False, stop=False,
                                             perf_mode=DR)
                        else:
                            nc.tensor.matmul(o_ps[:OUT, 256:512], vv,
                                             ex[:, s2:s2 + 2, 256:512],
                                             start=False, stop=(p == 3),
                                             perf_mode=DR)

                    for p in range(4):
                        av_pair(p)
                    den = sp.tile([1, 512], F32, tag="rr", name=f"r{l}_{h16}")
                    nc.scalar.activation(den[:], o_ps[VOCAB:OUT, :], AF.Ln)
                    nc.vector.tensor_copy(o_sb[off:off + D, hp, :], o_ps[:D, :])
                    pending.append((h16, den))
                    while len(pending) > 1:
                        _normalize(pending.pop(0))

                for tkt in range(8):
                    v_tile(tkt)
                exs = {0: head_scores(0)}
                for h16 in range(16):
                    if h16 + 1 < 16:
                        exs[h16 + 1] = head_scores(h16 + 1)
                    head_avs(h16, exs.pop(h16))
                while pending:
                    _normalize(pending.pop(0))

                sc_at.__exit__(None, None, None)
                sc_wo = nc.named_scope(f"wo.{l}"); sc_wo.__enter__()
                # ===== Wo + residual =====
                for cot in range(8):
                    wo_sb = w512p.tile([128, 8, 128], F8, tag="w5",
                                       name=f"wo{l}_{cot}")
                    nc.sync.dma_start(wo_sb[:], wo_ext[l, cot])
                    xo_ps = ps512.tile([128, 512], F32, tag="p5",
                                       name=f"xo{l}{cot}")
                    for hdp in range(4):
                        nc.tensor.matmul(xo_ps[:],
                                         wo_sb[:, 2 * hdp:2 * hdp + 2, :],
                                         o_f8[:, 2 * hdp:2 * hdp + 2, :],
                                         start=(hdp == 0), stop=(hdp == 3),
                                         perf_mode=DR)
                    xo_sb = sp.tile([128, 512], F32, tag="xo", name=f"xs{l}{cot}")
                    nc.scalar.activation(xo_sb[:], xo_ps[:], AF.Identity,
                                         bias=bo_sb[:, l, cot, :],
                                         scale=1.0 / SW)
                    nc.vector.tensor_add(x_sb[:, cot, :], x_sb[:, cot, :],
                                         xo_sb[:])

                sc_wo.__exit__(None, None, None)
                sc_n2 = nc.named_scope(f"n2.{l}"); sc_n2.__enter__()
                # ===== norm2 -> h2 (fp8) =====
                h2_sb = bigp.tile([128, 8, 512], F8, tag="big", name=f"h2_{l}")
                rbc2 = rms_rbc(f"b{l}")
                for ct in range(8):
                    nc.vector.tensor_tensor(h2_sb[:, ct, :], x_sb[:, ct, :],
                                            rbc2[:], OP.mult)

                sc_n2.__exit__(None, None, None)
                sc_ff = nc.named_scope(f"ff.{l}"); sc_ff.__enter__()
                # ===== FFN (ft chunks of 4; fp8 DoubleRow both matmuls) =====
                def emit_w2(chunk, u_prev, w2c):
                    for cot in range(8):
                        y_ps = ps128o.tile([128, 512], F32, tag="po",
                                          name=f"y{l}{chunk}{cot}")
                        for p in range(2):
                            nc.tensor.matmul(y_ps[:], w2c[p][:, :, cot, :],
                                             u_prev[:, 2 * p:2 * p + 2, :],
                                             start=(p == 0), stop=(p == 1),
                                             perf_mode=DR)
                        nc.vector.scalar_tensor_tensor(
                            x_sb[:, cot, :], y_ps[:], 1.0 / SW,
                            x_sb[:, cot, :], OP.mult, OP.add)
                        if chunk == 0:
                            # b2 folded in early: runs on scalar while later
                            # chunks' matmuls stream, off the layer-end path
                            nc.scalar.add(x_sb[:, cot, :], x_sb[:, cot, :],
                                          b2_sb[:, l, cot, :])

                prev = None
                for chunk in range(8):
                    u_sb = sp.tile([128, 4, 512], F8, tag="u",
                                   name=f"u{l}_{chunk}")
                    w2c = []
                    for fi in range(4):
                        ft = chunk * 4 + fi
                        w1_sb = w512p.tile([128, 8, 128], F8, tag="w5",
                                           name=f"w1_{l}_{ft}")
                        nc.sync.dma_start(w1_sb[:], w1_ext[l, ft])
                        if fi % 2 == 0:
                            w2_sb = w2p.tile([128, 2, 8, 128], F8, tag="w2",
                                             name=f"w2_{l}_{chunk}_{fi // 2}")
                            nc.sync.dma_start(w2_sb[:],
                                              w2_ext[l, chunk * 2 + fi // 2])
                            w2c.append(w2_sb)
                        u_pool, u_tag = ((ps512, "p5") if fi % 2 == 0
                                         else (ps128, "pk"))
                        u_ps = u_pool.tile([128, 512], F32, tag=u_tag,
                                           name=f"u{l}{ft}")
                        for cp in range(4):
                            nc.tensor.matmul(u_ps[:],
                                             w1_sb[:, 2 * cp:2 * cp + 2, :],
                                             h2_sb[:, 2 * cp:2 * cp + 2, :],
                                             start=(cp == 0), stop=(cp == 3),
                                             perf_mode=DR)
                        nc.scalar.activation(u_sb[:, fi, :], u_ps[:], AF.Gelu,
                                             bias=b1_sb[:, l, ft, :],
                                             scale=1.0 / SW)
                    if prev is not None:
                        emit_w2(*prev)
                    prev = (chunk, u_sb, w2c)
                emit_w2(*prev)


                if debug and l == 0:
                    nc.sync.dma_start(dbg_h[:], h_str[:])
                    nc.sync.dma_start(dbg_k[:], k_sb[:])
                    nc.sync.dma_start(dbg_q[:], q_sb[:])
                    nc.sync.dma_start(dbg_v[:], v_sb[:])
                    nc.sync.dma_start(dbg_o[:], o_f8[:])
                    nc.sync.dma_start(dbg_u[:], u_sb[:])
                    nc.sync.dma_start(dbg_x2[:], x_sb[:])

                sc_ff.__exit__(None, None, None)

            # ===== lm head + log_softmax / log_sigmoid =====
            for tlt in range(4):
                lg = ps512.tile([128, OUT], F32, tag="p5", name=f"lg{tlt}")
                for ct in range(8):
                    xr = sp.tile([128, 128], BF16, tag="xr", name=f"xr{tlt}_{ct}")
                    nc.scalar.copy(xr[:], x_sb[:, ct, tlt * 128:(tlt + 1) * 128])
                    nc.tensor.matmul(lg[:], xr[:], lmw_sb[:, ct, :],
                                     start=(ct == 0), stop=(ct == 7))
                lgb = sp.tile([128, OUT], F32, tag="lgb", name=f"lgb{tlt}")
                nc.vector.tensor_add(lgb[:], lg[:], lmb_sb[:])
                m = sp.tile([128, 1], F32, tag="m", name=f"m{tlt}")
                nc.vector.reduce_max(m[:], lgb[:, 0:VOCAB], axis=AX.X)
                nm = sp.tile([128, 1], F32, tag="nm", name=f"nm{tlt}")
                nc.scalar.mul(nm[:], m[:], -1.0)
                e = sp.tile([128, VOCAB], F32, tag="e", name=f"e{tlt}")
                es = sp.tile([128, 1], F32, tag="es", name=f"es{tlt}")
                nc.scalar.activation(e[:], lgb[:, 0:VOCAB], AF.Exp, bias=nm[:],
                                     accum_out=es[:])
                lse = sp.tile([128, 1], F32, tag="lse", name=f"lse{tlt}")
                nc.scalar.activation(lse[:], es[:], AF.Ln)
                bt = sp.tile([128, 1], F32, tag="bt", name=f"bt{tlt}")
                nc.vector.tensor_tensor(bt[:], nm[:], lse[:], OP.subtract)
                outt = sp.tile([128, OUT], F32, tag="outt", name=f"ot{tlt}")
                nc.scalar.activation(outt[:, 0:VOCAB], lgb[:, 0:VOCAB],
                                     AF.Identity, bias=bt[:])
                sg = sp.tile([128, 1], F32, tag="sg", name=f"sg{tlt}")
                nc.scalar.activation(sg[:], lgb[:, VOCAB:OUT], AF.Sigmoid)
                nc.scalar.activation(outt[:, VOCAB:OUT], sg[:], AF.Ln)
                nc.sync.dma_start(out_ext[tlt * 128:(tlt + 1) * 128, :], outt[:])

    _split_sync_waits(nc)
    return nc


# ---------------------------------------------------------------------------
# host-side preparation
# ---------------------------------------------------------------------------
def _own_rows(core):
    return np.concatenate(
        [np.arange(b * 128, (b + 1) * 128) for b in OWN_BLOCKS[core % 2]]
    )


def _bf(a):
    return np.asarray(a, dtype=ml_dtypes.bfloat16)


def _f8(a, s):
    return np.clip(np.asarray(a, np.float32) * s, -240.0, 240.0).astype(
        ml_dtypes.float8_e4m3fn
    )


def _f32(a):
    return np.ascontiguousarray(a, dtype=np.float32)


def _prep(inputs):
    acts = np.asarray(inputs["acts"])
    durations = _f32(inputs["durations"])
    emb_table = _f32(inputs["emb_table"])
    pos_table = _f32(inputs["pos_table"])
    Wq, Wk, Wv = (_f32(inputs[k]) for k in ("Wq", "Wk", "Wv"))
    Wo, bo = _f32(inputs["Wo"]), _f32(inputs["bo"])
    W1, b1 = _f32(inputs["W1"]), _f32(inputs["b1"])
    W2, b2 = _f32(inputs["W2"]), _f32(inputs["b2"])
    g1, g2 = _f32(inputs["g1"]), _f32(inputs["g2"])
    lm_W, lm_b = _f32(inputs["lm_W"]), _f32(inputs["lm_b"])

    # fold g1 into Wq/Wk/Wv (q also gets the D^-0.5 score scale), g2 into W1
    Wq_eff = Wq * g1[:, None, :, None] * (D ** -0.5)
    Wk_eff = Wk * g1[:, None, :, None]
    Wv_eff = Wv * g1[:, None, :, None]
    W1_eff = W1 * g2[:, :, None]

    def qk_arr(A, s):  # [L,H,C,D] -> [L, hp, cp, ct, m], fp8 scaled by s
        A2 = A.transpose(0, 2, 1, 3).reshape(L, C, H * D)
        return _f8(A2.reshape(L, 8, 128, 8, 128).transpose(0, 3, 2, 1, 4), s)

    shared = {
        "aug_table": None, "ones_col": _bf(np.ones((128, 1))),
        "ones_row": _f32(np.ones((1, 128))),
        "ones_row_bf": _bf(np.ones((1, 128))),
        "Wq_arr": qk_arr(Wq_eff, SWQ), "Wk_arr": qk_arr(Wk_eff, SW),
        "Wv_arr": _f8(Wv_eff.transpose(0, 2, 1, 3).reshape(L, C, H * D)
                      .reshape(L, 8, 128, 2, 512).transpose(0, 2, 3, 1, 4), SW),
        "Wo_arr": _f8(Wo.reshape(L, 8, 128, 8, 128).transpose(0, 3, 2, 1, 4), SW),
        "W1_arr": _f8(W1_eff.reshape(L, 8, 128, 32, 128)
                      .transpose(0, 3, 2, 1, 4), SW),
        "W2_arr": _f8(W2.reshape(L, 16, 2, 128, 8, 128)
                      .transpose(0, 1, 3, 2, 4, 5), SW),
        "bo_fm": np.ascontiguousarray(
            bo.reshape(L, 8, 128).transpose(2, 0, 1))[..., None],
        "b1_fm": np.ascontiguousarray(
            b1.reshape(L, 32, 128).transpose(2, 0, 1))[..., None],
        "b2_fm": np.ascontiguousarray(
            b2.reshape(L, 8, 128).transpose(2, 0, 1))[..., None],
        "lmW_arr": _bf(lm_W.reshape(8, 128, OUT).transpose(1, 0, 2)),
        "lmb_bc": _f32(np.tile(lm_b[None, :], (128, 1))),
    }
    aug = np.zeros((OUT, C), np.float32)
    aug[:VOCAB, : C - 1] = emb_table
    aug[VOCAB, C - 1] = 1.0
    shared["aug_table"] = _f32(aug.reshape(OUT, 8, 128))

    in_maps = []
    for core in range(8):
        b, half = core // 2, core % 2
        rows = _own_rows(core)
        oh = np.zeros((OUT, 512), np.float32)
        oh[acts[b, rows], np.arange(512)] = 1.0
        oh[VOCAB, :] = durations[b, rows]
        pos = pos_table[rows].T.reshape(8, 128, 512).transpose(1, 0, 2)
        masks = np.ones((8, 128, 512), np.float32)
        for s in range(8):
            gk = s * 128
            ii = gk + np.arange(128)[:, None]
            for j in range(4):
                gq = OWN_BLOCKS[half][j] * 128
                jj = gq + np.arange(128)[None, :]
                masks[s, :, j * 128:(j + 1) * 128] = (ii <= jj)
        m = dict(shared)
        m["onehot_t"] = _f32(oh)
        m["pos_fm"] = _bf(pos)
        m["masks"] = _bf(masks.transpose(1, 0, 2))
        in_maps.append(m)
    return in_maps


LAST_EXEC_NS = [None]
LAST_SCOPES = [None]


def kernel(**inputs) -> np.ndarray:
    nc = build_graph()
    in_maps = _prep(inputs)
    trace = bool(int(os.environ.get("KERNEL_TRACE", "0")))
    res = bass_utils.run_bass_kernel_spmd(
        nc, in_maps, list(range(8)), trace=trace,
        trace_cores=[0] if trace else None,
    )
    LAST_EXEC_NS[0] = res.exec_time_ns
    LAST_SCOPES[0] = res.per_core_scope_times
    if trace and res.instructions_and_trace:
        print("trace path:", res.instructions_and_trace[1])
    full = np.zeros((B, T, OUT), np.float32)
    for core in range(8):
        full[core // 2, _own_rows(core)] = res.results[core]["out"]
    return full



# revision 45
# speedup vs baseline: 1.1950x; 1.1649x over previous
"""Trainium2 Bass kernel for nn_AttentionDecoder_82738249990894 (B=4, T=1024,
C=1024, H=16, D=64, F=4096, L=4, vocab 64+1 outputs).

Sharding: sequence-split data parallel over 8 cores.  Core c handles batch
b = c//2, sequence half = c%2.  Balanced causal split: half0 owns global
128-row blocks [0,1,6,7], half1 owns [2,3,4,5] (equal attention work: both
see 18 causal k-tiles).  Per layer the pair exchanges rmsnorm'd activations
(bf16, pairwise AllGather, ~1MB) and each core recomputes k/v for all 1024
tokens locally.  No other communication.

SPMD uniformity: one graph runs on all 8 cores, so the key/value strip is
kept in GLOBAL token order (the AllGather return scatters both pair slots
to fixed global positions) and every local q-tile j computes scores against
the union visibility vis_u=[3,4,7,8] k-tiles; per-core 0/1 masks (input
data) encode causality and half-dependent visibility.

Matmul dtypes: fp8e4m3 with DoubleRow perf mode (2 k-tiles per pass, 2x
PE throughput) for the qkv/Wo/W1/W2/AV matmuls; weights are pre-scaled
(x64, x512 for Wq) into fp8's dynamic range on the host and the inverse
scale is folded into the psum->sbuf copy or activation that follows.
Scores stay bf16 (64-deep contraction, DoubleRow inapplicable).
Residual x stays fp32.  Softmax skips max-subtraction (scores are O(1);
fp32 psum exp is safe) and gets denominators free via a ones-column
appended to v; normalization is deferred to after the AV matmul.
"""
import os
import sys
import types

sys.path.insert(0, "/opt/trn_rl_repo")

import numpy as np
import ml_dtypes

import antenv

if not hasattr(antenv, "axon_hooks"):
    _mod = types.ModuleType("antenv.axon_hooks")
    _mod._hook = None
    _mod.set_axon_ntff_profile_hook = lambda h: setattr(_mod, "_hook", h)
    _mod.get_axon_ntff_profile_hook = lambda: _mod._hook
    sys.modules["antenv.axon_hooks"] = _mod
    antenv.axon_hooks = _mod
    try:
        from trn_agent_boot.trn_boot import _ntff_profile_via_ctypes

        _mod.set_axon_ntff_profile_hook(
            _ntff_profile_via_ctypes("/opt/axon/libaxon_pjrt.so")
        )
    except Exception:
        pass

import concourse.bass as bass
import concourse.mybir as mybir
import concourse.tile as tile
from concourse import bass_utils

bass_utils.upload_artifacts = lambda tmpdir: "local://" + tmpdir
try:
    from concourse import tile_utils as _tu

    _tu.max_sbuf_usage = 206 * 1024
except Exception:
    pass

F32 = mybir.dt.float32
F32R = mybir.dt.float32r
BF16 = mybir.dt.bfloat16
F8 = mybir.dt.float8e4
DR = mybir.MatmulPerfMode.DoubleRow
AF = mybir.ActivationFunctionType
OP = mybir.AluOpType
AX = mybir.AxisListType

SW = 64.0     # fp8 weight pre-scale (Wk/Wv/Wo/W1/W2)
SWQ = 512.0   # Wq pre-scale (D^-0.5 folded in makes it 8x smaller)

B, T, C, H, D, F, L = 4, 1024, 1024, 16, 64, 4096, 4
VOCAB, OUT = 64, 65
EPS = float(np.finfo(np.float32).eps)
RG = [[0, 1], [2, 3], [4, 5], [6, 7]]
OWN_BLOCKS = {0: [0, 1, 6, 7], 1: [2, 3, 4, 5]}
VIS_U = [3, 4, 7, 8]          # union visible k-tiles per local q-tile
N_MASK = 3                    # last 3 visible slots carry a mask

_wsplit_ctr = [0]


def _split_sync_waits(nc):
    """This walrus build allows one sync-wait per instruction; hoist extras
    onto injected same-engine NoOps."""
    for f in nc.m.functions:
        for bb in f.blocks:
            out = []
            changed = False
            for inst in bb.instructions:
                si = getattr(inst, "sync_info", None)
                if si is not None and si.on_wait is not None and len(si.on_wait) > 1:
                    waits = list(si.on_wait)
                    for w in waits[:-1]:
                        _wsplit_ctr[0] += 1
                        n = mybir.InstNoOp(
                            name=f"WSPLIT-{_wsplit_ctr[0]}", ins=[], outs=[]
                        )
                        n.engine = inst.engine
                        n.sync_info = mybir.SyncInfo(on_wait=[w], on_update=[])
                        out.append(n)
                    inst.sync_info = mybir.SyncInfo(
                        on_wait=[waits[-1]], on_update=list(si.on_update)
                    )
                    changed = True
                out.append(inst)
            if changed:
                bb.instructions[:] = out


def build_graph():
    nc = bass.Bass()
    dp = nc.declare_dram_parameter
    onehot_ext = dp("onehot_t", [OUT, 512], F32R, isOutput=False)
    pos_ext = dp("pos_fm", [128, 8, 512], BF16, isOutput=False)
    aug_ext = dp("aug_table", [OUT, 8, 128], F32R, isOutput=False)
    mask_ext = dp("masks", [128, 8, 512], BF16, isOutput=False)
    onescol_ext = dp("ones_col", [128, 1], BF16, isOutput=False)
    onesrow_ext = dp("ones_row", [1, 128], F32R, isOutput=False)
    onesrowb_ext = dp("ones_row_bf", [1, 128], BF16, isOutput=False)
    wq_ext = dp("Wq_arr", [L, 8, 128, 8, 128], F8, isOutput=False)
    wk_ext = dp("Wk_arr", [L, 8, 128, 8, 128], F8, isOutput=False)
    wv_ext = dp("Wv_arr", [L, 128, 2, 8, 512], F8, isOutput=False)
    wo_ext = dp("Wo_arr", [L, 8, 128, 8, 128], F8, isOutput=False)
    w1_ext = dp("W1_arr", [L, 32, 128, 8, 128], F8, isOutput=False)
    w2_ext = dp("W2_arr", [L, 16, 128, 2, 8, 128], F8, isOutput=False)
    bo_ext = dp("bo_fm", [128, L, 8, 1], F32, isOutput=False)
    b1_ext = dp("b1_fm", [128, L, 32, 1], F32, isOutput=False)
    b2_ext = dp("b2_fm", [128, L, 8, 1], F32, isOutput=False)
    lmw_ext = dp("lmW_arr", [128, 8, OUT], BF16, isOutput=False)
    lmb_ext = dp("lmb_bc", [128, OUT], F32, isOutput=False)
    out_ext = dp("out", [512, OUT], F32, isOutput=True)
    debug = bool(int(os.environ.get("KERNEL_DEBUG", "0")))
    if debug:
        dbg_h = dp("dbg_h", [128, 8, 1024], F8, isOutput=True)
        dbg_k = dp("dbg_k", [128, 8, 1024], BF16, isOutput=True)
        dbg_q = dp("dbg_q", [128, 8, 512], BF16, isOutput=True)
        dbg_v = dp("dbg_v", [128, 8, 16, 66], F8, isOutput=True)
        dbg_o = dp("dbg_o", [128, 8, 512], F8, isOutput=True)
        dbg_u = dp("dbg_u", [128, 4, 512], F8, isOutput=True)
        dbg_x2 = dp("dbg_x2", [128, 8, 512], F32, isOutput=True)

    with tile.TileContext(nc) as tc:
        nc_lp = nc.allow_low_precision(reason="bf16 attention path is intentional")
        nc_lp.__enter__()
        with (
            tc.tile_pool(name="persist", bufs=1) as pp,
            tc.tile_pool(name="scratch", bufs=2) as sp,
            tc.tile_pool(name="wqk", bufs=3) as wqkp,
            tc.tile_pool(name="w512", bufs=3) as w512p,
            tc.tile_pool(name="w2p", bufs=5) as w2p,
            tc.tile_pool(name="bigp", bufs=1) as bigp,
            tc.tile_pool(name="wvp", bufs=1) as wvp,
            tc.tile_pool(name="ps512", bufs=2, space="PSUM") as ps512,
            tc.tile_pool(name="ps128", bufs=2, space="PSUM") as ps128,
            tc.tile_pool(name="ps128o", bufs=2, space="PSUM") as ps128o,
            tc.tile_pool(name="pswide", bufs=1, space="PSUM") as pswide,
            tc.tile_pool(name="dram", bufs=2, space="DRAM") as dram,
        ):
            # ---- constants ----
            ones_col = pp.tile([128, 1], BF16)
            ones_row = pp.tile([1, 128], F32R)
            ones_row_bf = pp.tile([1, 128], BF16)
            aug_sb = pp.tile([OUT, 8, 128], F32R)
            onehot_sb = pp.tile([OUT, 512], F32R)
            mask_sb = pp.tile([128, 8, 512], BF16)
            lmw_sb = pp.tile([128, 8, OUT], BF16)
            lmb_sb = pp.tile([128, OUT], F32)
            bo_sb = pp.tile([128, L, 8, 1], F32)
            b1_sb = pp.tile([128, L, 32, 1], F32)
            b2_sb = pp.tile([128, L, 8, 1], F32)
            nc.sync.dma_start(aug_sb[:], aug_ext[:])
            nc.sync.dma_start(onehot_sb[:], onehot_ext[:])
            nc.sync.dma_start(ones_col[:], onescol_ext[:])
            nc.sync.dma_start(ones_row[:], onesrow_ext[:])
            nc.sync.dma_start(ones_row_bf[:], onesrowb_ext[:])

            # tiny warm-up collective: absorbs first-use CC latency while
            # the tensor engine runs the embedding
            cw_in = dram.tile([1, 64], F32, tag="cwi", name="cwi")
            cw_out = dram.tile([2, 64], F32, tag="cwo", name="cwo")
            cw_sb = pp.tile([1, 64], F32)
            nc.gpsimd.memset(cw_sb[:], 0.0)
            nc.sync.dma_start(cw_in[:], cw_sb[:])
            nc.gpsimd.collective_compute(
                "AllGather", OP.bypass,
                ins=[cw_in[:].opt()],
                outs=[cw_out[:].opt()],
                replica_groups=RG,
            )

            nc.sync.dma_start(mask_sb[:], mask_ext[:])
            nc.sync.dma_start(lmw_sb[:], lmw_ext[:])
            nc.sync.dma_start(lmb_sb[:], lmb_ext[:])
            nc.sync.dma_start(bo_sb[:], bo_ext[:])
            nc.sync.dma_start(b1_sb[:], b1_ext[:])
            nc.sync.dma_start(b2_sb[:], b2_ext[:])

            eps_sb = pp.tile([128, 1], F32)
            nc.gpsimd.memset(eps_sb[:], EPS)

            # ---- persistent activations ----
            x_sb = pp.tile([128, 8, 512], F32)       # residual (feature-major)
            h_own = pp.tile([128, 8, 512], F8)       # norm'd own tokens
            h_str = pp.tile([128, 8, 1024], F8)      # norm'd pair, global order
            q_sb = pp.tile([128, 8, 512], BF16)      # [2h*64, hp, local t]
            k_sb = pp.tile([128, 8, 1024], BF16)     # [2h*64, hp, global t]
            # inner dim padded 65->66 so every fp8 slice lands on an even
            # byte offset (odd SBUF offsets break engine reads)
            v_sb = pp.tile([128, 8, 16, 66], F8)     # [tk, tkt, head, d+1+pad]
            o_sb = pp.tile([128, 8, 512], BF16)      # attn out, unnormalized
            o_f8 = pp.tile([128, 8, 512], F8)        # normalized attn out
            # pos and per-layer h2 share one big slot (disjoint lifetimes)
            pos_sb = bigp.tile([128, 8, 512], BF16, tag="big", name="pos")
            nc.sync.dma_start(pos_sb[:], pos_ext[:])

            # ---- embedding: x = onehot @ aug_table + pos ----
            sc_emb = nc.named_scope("emb"); sc_emb.__enter__()
            for ct in range(8):
                emb_ps = ps512.tile([128, 512], F32, tag="p5", name=f"emb{ct}")
                nc.tensor.matmul(emb_ps[:], aug_sb[:, ct, :], onehot_sb[:],
                                 start=True, stop=True)
                nc.vector.tensor_add(x_sb[:, ct, :], emb_ps[:], pos_sb[:, ct, :])

            sc_emb.__exit__(None, None, None)

            def rms_rbc(tag):
                ssum = ps512.tile([128, 512], F32, tag="p5", name=f"ss{tag}")
                for ct in range(8):
                    xsq = sp.tile([128, 512], BF16, tag="xsq", name=f"xq{tag}{ct}")
                    if ct % 2 == 0:
                        nc.scalar.activation(xsq[:], x_sb[:, ct, :], AF.Square)
                    else:
                        nc.vector.tensor_tensor(xsq[:], x_sb[:, ct, :],
                                                x_sb[:, ct, :], OP.mult)
                    nc.tensor.matmul(ssum[:1, :], ones_col[:], xsq[:],
                                     start=(ct == 0), stop=(ct == 7))
                lnv = sp.tile([1, 512], F32, tag="lnv", name=f"lv{tag}", bufs=1)
                nc.scalar.activation(lnv[:], ssum[:1, :], AF.Ln,
                                     bias=eps_sb[:1, :], scale=1.0 / C)
                rstd = sp.tile([1, 512], F32R, tag="sqv", name=f"sv{tag}",
                               bufs=1)
                nc.scalar.activation(rstd[:], lnv[:], AF.Exp, scale=-0.5)
                rbc = ps512.tile([128, 512], F32, tag="p5", name=f"rb{tag}")
                nc.tensor.matmul(rbc[:], ones_row[:], rstd[:], start=True,
                                 stop=True)
                return rbc

            for l in range(L):
                sc_n1 = nc.named_scope(f"n1.{l}"); sc_n1.__enter__()
                # ===== norm1 -> h_own =====
                rbc = rms_rbc(f"a{l}")
                for ct in range(8):
                    nc.vector.tensor_tensor(h_own[:, ct, :], x_sb[:, ct, :],
                                            rbc[:], OP.mult)

                # ===== pair exchange (AllGather, fp8) =====
                bounce = dram.tile([8, 128, 512], F8, tag="agin", name=f"agi{l}")
                for ct in range(8):
                    nc.sync.dma_start(bounce[ct], h_own[:, ct, :])
                gath = dram.tile([2, 8, 128, 512], F8, tag="agout",
                                 name=f"ago{l}")
                nc.gpsimd.collective_compute(
                    "AllGather", OP.bypass,
                    ins=[bounce[:].opt()],
                    outs=[gath[:].opt()],
                    replica_groups=RG,
                )

                sc_n1.__exit__(None, None, None)
                sc_q = nc.named_scope(f"q.{l}"); sc_q.__enter__()
                # Wv for this layer (no AG dependency -> overlaps exchange)
                wv_sb = wvp.tile([128, 2, 8, 512], F8, tag="wv", name=f"wv{l}")
                nc.sync.dma_start(wv_sb[:], wv_ext[l])

                # ===== q from h_own (overlaps AG) =====
                for hp in range(8):
                    wq_sb = wqkp.tile([128, 8, 128], F8, tag="wqk",
                                      name=f"wq{l}_{hp}")
                    nc.sync.dma_start(wq_sb[:], wq_ext[l, hp])
                    q_ps = ps512.tile([128, 512], F32, tag="p5", name=f"q{l}{hp}")
                    for cp in range(4):
                        nc.tensor.matmul(q_ps[:], wq_sb[:, 2 * cp:2 * cp + 2, :],
                                         h_own[:, 2 * cp:2 * cp + 2, :],
                                         start=(cp == 0), stop=(cp == 3),
                                         perf_mode=DR)
                    nc.vector.tensor_scalar_mul(q_sb[:, hp, :], q_ps[:],
                                                1.0 / SWQ)

                sc_q.__exit__(None, None, None)
                sc_kv = nc.named_scope(f"kv.{l}"); sc_kv.__enter__()
                # ===== scatter AG result into global-order strip =====
                # slot0 = half0 local blocks -> global [0,1,6,7]
                # slot1 = half1 local blocks -> global [2,3,4,5]
                for ct in range(8):
                    nc.sync.dma_start(h_str[:, ct, 0:256], gath[0, ct, :, 0:256])
                    nc.sync.dma_start(h_str[:, ct, 768:1024], gath[0, ct, :, 256:512])
                    nc.sync.dma_start(h_str[:, ct, 256:768], gath[1, ct])

                # ===== k over the strip =====
                for hp in range(8):
                    wk_sb = wqkp.tile([128, 8, 128], F8, tag="wqk",
                                      name=f"wk{l}_{hp}")
                    nc.sync.dma_start(wk_sb[:], wk_ext[l, hp])
                    for half in range(2):
                        k_pool, k_tag = ((ps512, "p5") if half == 0
                                         else (ps128, "pk"))
                        k_ps = k_pool.tile([128, 512], F32, tag=k_tag,
                                           name=f"k{l}{hp}{half}")
                        for cp in range(4):
                            nc.tensor.matmul(
                                k_ps[:], wk_sb[:, 2 * cp:2 * cp + 2, :],
                                h_str[:, 2 * cp:2 * cp + 2,
                                      half * 512:(half + 1) * 512],
                                start=(cp == 0), stop=(cp == 3), perf_mode=DR)
                        nc.vector.tensor_scalar_mul(
                            k_sb[:, hp, half * 512:(half + 1) * 512], k_ps[:],
                            1.0 / SW)

                sc_kv.__exit__(None, None, None)
                sc_at = nc.named_scope(f"at.{l}"); sc_at.__enter__()
                # ===== v tiles interleaved with per-head scores: the
                # scalar exp stream starts while the tensor engine is still
                # on v matmuls =====
                pending = []

                def _normalize(pend):
                    ph, pden = pend
                    php, poff = ph // 2, (ph % 2) * D
                    r = sp.tile([1, 512], BF16, tag="rex", name=f"re{l}_{ph}")
                    nc.scalar.activation(r[:], pden[:], AF.Exp, scale=-1.0)
                    rb_ps = ps128.tile([128, 512], F32, tag="pk",
                                       name=f"rb{l}_{ph}")
                    nc.tensor.matmul(rb_ps[poff:poff + D, :],
                                     ones_row_bf[:, 0:D],
                                     r[:], start=True, stop=True)
                    nc.vector.tensor_tensor(
                        o_f8[poff:poff + D, php, :], o_sb[poff:poff + D, php, :],
                        rb_ps[poff:poff + D, :], OP.mult)

                def v_tile(tkt):
                    for vh in range(2):
                        v_pool, v_tag = ((ps512, "p5") if vh == 0
                                         else (ps128, "pk"))
                        v_ps = v_pool.tile([128, 512], F32, tag=v_tag,
                                           name=f"v{l}{tkt}{vh}")
                        for cp in range(4):
                            nc.tensor.matmul(
                                v_ps[:],
                                h_str[:, 2 * cp:2 * cp + 2,
                                      tkt * 128:(tkt + 1) * 128],
                                wv_sb[:, vh, 2 * cp:2 * cp + 2, :],
                                start=(cp == 0), stop=(cp == 3), perf_mode=DR)
                        nc.vector.tensor_scalar_mul(
                            v_sb[:, tkt, 8 * vh:8 * vh + 8, 0:D],
                            v_ps[:].rearrange("p (q d) -> p q d", d=D),
                            1.0 / SW)
                    nc.gpsimd.memset(v_sb[:, tkt, :, D:66], 1.0)

                def head_scores(h16):
                    hp, off = h16 // 2, (h16 % 2) * D
                    ex = sp.tile([128, 8, 512], F8, tag="expA", bufs=3,
                                 name=f"ex{l}_{h16}")

                    def score_wide2(sp0):
                        # two wide k-tiles into one dedicated 2-bank psum
                        # tile; one [128,1024] exp drains both (halves the
                        # scalar ACTIVATE overhead on the pacing engine)
                        w_ps = pswide.tile([128, 2, 512], F32, tag="pw",
                                           name=f"s{l}_{h16}_{sp0}")
                        for j in range(2):
                            s = sp0 + j
                            nc.tensor.matmul(
                                w_ps[:, j, :],
                                k_sb[off:off + D, hp, s * 128:(s + 1) * 128],
                                q_sb[off:off + D, hp, :],
                                start=True, stop=True)
                        nc.scalar.activation(ex[:, sp0:sp0 + 2, :], w_ps[:],
                                             AF.Exp)

                    def score_narrow(pi):
                        s_pool, s_tag = ((ps512, "p5") if pi == 0
                                         else (ps128, "pk"))
                        n_ps = s_pool.tile([128, 512], F32, tag=s_tag,
                                           name=f"n{l}_{h16}_{pi}")
                        for j in range(2):
                            s = 4 + 2 * pi + j
                            nc.tensor.matmul(
                                n_ps[:, 256 * j:256 * (j + 1)],
                                k_sb[off:off + D, hp, s * 128:(s + 1) * 128],
                                q_sb[off:off + D, hp, 256:512],
                                start=True, stop=True)
                        s0 = 4 + 2 * pi
                        nc.scalar.activation(
                            ex[:, s0:s0 + 2, 256:512],
                            n_ps[:].rearrange("p (s m) -> p s m", s=2),
                            AF.Exp)

                    def mask2(s0, c0):
                        nc.vector.tensor_tensor(
                            ex[:, s0:s0 + 2, c0:c0 + 256],
                            ex[:, s0:s0 + 2, c0:c0 + 256],
                            mask_sb[:, s0:s0 + 2, c0:c0 + 256], OP.mult)

                    score_wide2(0)
                    mask2(0, 0)
                    score_wide2(2)
                    mask2(2, 0)
                    score_narrow(0)
                    mask2(4, 256)
                    score_narrow(1)
                    mask2(6, 256)
                    return ex

                def head_avs(h16, ex):
                    hp, off = h16 // 2, (h16 % 2) * D
                    o_ps = ps128o.tile([128, 512], F32, tag="po",
                                       name=f"o{l}_{h16}")

                    def av_pair(p):
                        s2 = 2 * p
                        vv = v_sb[:, s2:s2 + 2, h16, 0:OUT]
                        if p == 0:
                            nc.tensor.matmul(o_ps[:OUT, :], vv,
                                             ex[:, s2:s2 + 2, :],
                                             start=True, stop=False,
                                             perf_mode=DR)
                        elif p == 1:
                            nc.tensor.matmul(o_ps[:OUT, 0:256], vv,
                                             ex[:, s2:s2 + 2, 0:256],
                                             start=False, stop=True,
                                             perf_mode=DR)
                            nc.tensor.matmul(o_ps[:OUT, 256:512], vv,
                                             ex[:, s2:s2 + 2, 256:512],
                                             start=False, stop=False,
                                             perf_mode=DR)
                        else:
                            nc.tensor.matmul(o_ps[:OUT, 256:512], vv,
                                             ex[:, s2:s2 + 2, 256:512],
                                             start=False, stop=(p == 3),
                                             perf_mode=DR)

                    for p in range(4):
                        av_pair(p)
                    den = sp.tile([1, 512], F32, tag="rr", name=f"r{l}_{h16}")
                    nc.scalar.activation(den[:], o_ps[VOCAB:OUT, :], AF.Ln)
                    nc.vector.tensor_copy(o_sb[off:off + D, hp, :], o_ps[:D, :])
                    pending.append((h16, den))
                    while len(pending) > 1:
                        _normalize(pending.pop(0))

                for tkt in range(8):
                    v_tile(tkt)
                exs = {0: head_scores(0)}
                for h16 in range(16):
                    if h16 + 1 < 16:
                        exs[h16 + 1] = head_scores(h16 + 1)
                    head_avs(h16, exs.pop(h16))
                while pending:
                    _normalize(pending.pop(0))

                sc_at.__exit__(None, None, None)
                sc_wo = nc.named_scope(f"wo.{l}"); sc_wo.__enter__()
                # ===== Wo + residual =====
                for cot in range(8):
                    wo_sb = w512p.tile([128, 8, 128], F8, tag="w5",
                                       name=f"wo{l}_{cot}")
                    nc.sync.dma_start(wo_sb[:], wo_ext[l, cot])
                    xo_ps = ps512.tile([128, 512], F32, tag="p5",
                                       name=f"xo{l}{cot}")
                    for hdp in range(4):
                        nc.tensor.matmul(xo_ps[:],
                                         wo_sb[:, 2 * hdp:2 * hdp + 2, :],
                                         o_f8[:, 2 * hdp:2 * hdp + 2, :],
                                         start=(hdp == 0), stop=(hdp == 3),
                                         perf_mode=DR)
                    xo_sb = sp.tile([128, 512], F32, tag="xo", name=f"xs{l}{cot}")
                    nc.scalar.activation(xo_sb[:], xo_ps[:], AF.Identity,
                                         bias=bo_sb[:, l, cot, :],
                                         scale=1.0 / SW)
                    nc.vector.tensor_add(x_sb[:, cot, :], x_sb[:, cot, :],
                                         xo_sb[:])

                sc_wo.__exit__(None, None, None)
                sc_n2 = nc.named_scope(f"n2.{l}"); sc_n2.__enter__()
                # ===== norm2 -> h2 (fp8) =====
                h2_sb = bigp.tile([128, 8, 512], F8, tag="big", name=f"h2_{l}")
                rbc2 = rms_rbc(f"b{l}")
                for ct in range(8):
                    nc.vector.tensor_tensor(h2_sb[:, ct, :], x_sb[:, ct, :],
                                            rbc2[:], OP.mult)

                sc_n2.__exit__(None, None, None)
                sc_ff = nc.named_scope(f"ff.{l}"); sc_ff.__enter__()
                # ===== FFN (ft chunks of 4; fp8 DoubleRow both matmuls) =====
                def emit_w2(chunk, u_prev, w2c):
                    for cot in range(8):
                        y_ps = ps128o.tile([128, 512], F32, tag="po",
                                          name=f"y{l}{chunk}{cot}")
                        for p in range(2):
                            nc.tensor.matmul(y_ps[:], w2c[p][:, :, cot, :],
                                             u_prev[:, 2 * p:2 * p + 2, :],
                                             start=(p == 0), stop=(p == 1),
                                             perf_mode=DR)
                        nc.vector.scalar_tensor_tensor(
                            x_sb[:, cot, :], y_ps[:], 1.0 / SW,
                            x_sb[:, cot, :], OP.mult, OP.add)
                        if chunk == 0:
                            # b2 folded in early: runs on scalar while later
                            # chunks' matmuls stream, off the layer-end path
                            nc.scalar.add(x_sb[:, cot, :], x_sb[:, cot, :],
                                          b2_sb[:, l, cot, :])

                prev = None
                for chunk in range(8):
                    u_sb = sp.tile([128, 4, 512], F8, tag="u",
                                   name=f"u{l}_{chunk}")
                    w2c = []
                    for fi in range(4):
                        ft = chunk * 4 + fi
                        w1_sb = w512p.tile([128, 8, 128], F8, tag="w5",
                                           name=f"w1_{l}_{ft}")
                        nc.sync.dma_start(w1_sb[:], w1_ext[l, ft])
                        if fi % 2 == 0:
                            w2_sb = w2p.tile([128, 2, 8, 128], F8, tag="w2",
                                             name=f"w2_{l}_{chunk}_{fi // 2}")
                            nc.sync.dma_start(w2_sb[:],
                                              w2_ext[l, chunk * 2 + fi // 2])
                            w2c.append(w2_sb)
                        u_pool, u_tag = ((ps512, "p5") if fi % 2 == 0
                                         else (ps128, "pk"))
                        u_ps = u_pool.tile([128, 512], F32, tag=u_tag,
                                           name=f"u{l}{ft}")
                        for cp in range(4):
                            nc.tensor.matmul(u_ps[:],
                                             w1_sb[:, 2 * cp:2 * cp + 2, :],
                                             h2_sb[:, 2 * cp:2 * cp + 2, :],
                                             start=(cp == 0), stop=(cp == 3),
                                             perf_mode=DR)
                        nc.scalar.activation(u_sb[:, fi, :], u_ps[:], AF.Gelu,
                                             bias=b1_sb[:, l, ft, :],
                                             scale=1.0 / SW)
                    if prev is not None:
                        emit_w2(*prev)
                    prev = (chunk, u_sb, w2c)
                emit_w2(*prev)


                if debug and l == 0:
                    nc.sync.dma_start(dbg_h[:], h_str[:])
                    nc.sync.dma_start(dbg_k[:], k_sb[:])
                    nc.sync.dma_start(dbg_q[:], q_sb[:])
                    nc.sync.dma_start(dbg_v[:], v_sb[:])
                    nc.sync.dma_start(dbg_o[:], o_f8[:])
                    nc.sync.dma_start(dbg_u[:], u_sb[:])
                    nc.sync.dma_start(dbg_x2[:], x_sb[:])

                sc_ff.__exit__(None, None, None)

            # ===== lm head + log_softmax / log_sigmoid =====
            for tlt in range(4):
                lg = ps512.tile([128, OUT], F32, tag="p5", name=f"lg{tlt}")
                for ct in range(8):
                    xr = sp.tile([128, 128], BF16, tag="xr", name=f"xr{tlt}_{ct}")
                    nc.scalar.copy(xr[:], x_sb[:, ct, tlt * 128:(tlt + 1) * 128])
                    nc.tensor.matmul(lg[:], xr[:], lmw_sb[:, ct, :],
                                     start=(ct == 0), stop=(ct == 7))
                lgb = sp.tile([128, OUT], F32, tag="lgb", name=f"lgb{tlt}")
                nc.vector.tensor_add(lgb[:], lg[:], lmb_sb[:])
                m = sp.tile([128, 1], F32, tag="m", name=f"m{tlt}")
                nc.vector.reduce_max(m[:], lgb[:, 0:VOCAB], axis=AX.X)
                nm = sp.tile([128, 1], F32, tag="nm", name=f"nm{tlt}")
                nc.scalar.mul(nm[:], m[:], -1.0)
                e = sp.tile([128, VOCAB], F32, tag="e", name=f"e{tlt}")
                es = sp.tile([128, 1], F32, tag="es", name=f"es{tlt}")
                nc.scalar.activation(e[:], lgb[:, 0:VOCAB], AF.Exp, bias=nm[:],
                                     accum_out=es[:])
                lse = sp.tile([128, 1], F32, tag="lse", name=f"lse{tlt}")
                nc.scalar.activation(lse[:], es[:], AF.Ln)
                bt = sp.tile([128, 1], F32, tag="bt", name=f"bt{tlt}")
                nc.vector.tensor_tensor(bt[:], nm[:], lse[:], OP.subtract)
                outt = sp.tile([128, OUT], F32, tag="outt", name=f"ot{tlt}")
                nc.scalar.activation(outt[:, 0:VOCAB], lgb[:, 0:VOCAB],
                                     AF.Identity, bias=bt[:])
                sg = sp.tile([128, 1], F32, tag="sg", name=f"sg{tlt}")
                nc.scalar.activation(sg[:], lgb[:, VOCAB:OUT], AF.Sigmoid)
                nc.scalar.activation(outt[:, VOCAB:OUT], sg[:], AF.Ln)
                nc.sync.dma_start(out_ext[tlt * 128:(tlt + 1) * 128, :], outt[:])

    _split_sync_waits(nc)
    return nc


# ---------------------------------------------------------------------------
# host-side preparation
# ---------------------------------------------------------------------------
def _own_rows(core):
    return np.concatenate(
        [np.arange(b * 128, (b + 1) * 128) for b in OWN_BLOCKS[core % 2]]
    )


def _bf(a):
    return np.asarray(a, dtype=ml_dtypes.bfloat16)


def _f8(a, s):
    return np.clip(np.asarray(a, np.float32) * s, -240.0, 240.0).astype(
        ml_dtypes.float8_e4m3fn
    )


def _f32(a):
    return np.ascontiguousarray(a, dtype=np.float32)


def _prep(inputs):
    acts = np.asarray(inputs["acts"])
    durations = _f32(inputs["durations"])
    emb_table = _f32(inputs["emb_table"])
    pos_table = _f32(inputs["pos_table"])
    Wq, Wk, Wv = (_f32(inputs[k]) for k in ("Wq", "Wk", "Wv"))
    Wo, bo = _f32(inputs["Wo"]), _f32(inputs["bo"])
    W1, b1 = _f32(inputs["W1"]), _f32(inputs["b1"])
    W2, b2 = _f32(inputs["W2"]), _f32(inputs["b2"])
    g1, g2 = _f32(inputs["g1"]), _f32(inputs["g2"])
    lm_W, lm_b = _f32(inputs["lm_W"]), _f32(inputs["lm_b"])

    # fold g1 into Wq/Wk/Wv (q also gets the D^-0.5 score scale), g2 into W1
    Wq_eff = Wq * g1[:, None, :, None] * (D ** -0.5)
    Wk_eff = Wk * g1[:, None, :, None]
    Wv_eff = Wv * g1[:, None, :, None]
    W1_eff = W1 * g2[:, :, None]

    def qk_arr(A, s):  # [L,H,C,D] -> [L, hp, cp, ct, m], fp8 scaled by s
        A2 = A.transpose(0, 2, 1, 3).reshape(L, C, H * D)
        return _f8(A2.reshape(L, 8, 128, 8, 128).transpose(0, 3, 2, 1, 4), s)

    shared = {
        "aug_table": None, "ones_col": _bf(np.ones((128, 1))),
        "ones_row": _f32(np.ones((1, 128))),
        "ones_row_bf": _bf(np.ones((1, 128))),
        "Wq_arr": qk_arr(Wq_eff, SWQ), "Wk_arr": qk_arr(Wk_eff, SW),
        "Wv_arr": _f8(Wv_eff.transpose(0, 2, 1, 3).reshape(L, C, H * D)
                      .reshape(L, 8, 128, 2, 512).transpose(0, 2, 3, 1, 4), SW),
        "Wo_arr": _f8(Wo.reshape(L, 8, 128, 8, 128).transpose(0, 3, 2, 1, 4), SW),
        "W1_arr": _f8(W1_eff.reshape(L, 8, 128, 32, 128)
                      .transpose(0, 3, 2, 1, 4), SW),
        "W2_arr": _f8(W2.reshape(L, 16, 2, 128, 8, 128)
                      .transpose(0, 1, 3, 2, 4, 5), SW),
        "bo_fm": np.ascontiguousarray(
            bo.reshape(L, 8, 128).transpose(2, 0, 1))[..., None],
        "b1_fm": np.ascontiguousarray(
            b1.reshape(L, 32, 128).transpose(2, 0, 1))[..., None],
        "b2_fm": np.ascontiguousarray(
            b2.reshape(L, 8, 128).transpose(2, 0, 1))[..., None],
        "lmW_arr": _bf(lm_W.reshape(8, 128, OUT).transpose(1, 0, 2)),
        "lmb_bc": _f32(np.tile(lm_b[None, :], (128, 1))),
    }
    aug = np.zeros((OUT, C), np.float32)
    aug[:VOCAB, : C - 1] = emb_table
    aug[VOCAB, C - 1] = 1.0
    shared["aug_table"] = _f32(aug.reshape(OUT, 8, 128))

    in_maps = []
    for core in range(8):
        b, half = core // 2, core % 2
        rows = _own_rows(core)
        oh = np.zeros((OUT, 512), np.float32)
        oh[acts[b, rows], np.arange(512)] = 1.0
        oh[VOCAB, :] = durations[b, rows]
        pos = pos_table[rows].T.reshape(8, 128, 512).transpose(1, 0, 2)
        masks = np.ones((8, 128, 512), np.float32)
        for s in range(8):
            gk = s * 128
            ii = gk + np.arange(128)[:, None]
            for j in range(4):
                gq = OWN_BLOCKS[half][j] * 128
                jj = gq + np.arange(128)[None, :]
                masks[s, :, j * 128:(j + 1) * 128] = (ii <= jj)
        m = dict(shared)
        m["onehot_t"] = _f32(oh)
        m["pos_fm"] = _bf(pos)
        m["masks"] = _bf(masks.transpose(1, 0, 2))
        in_maps.append(m)
    return in_maps


LAST_EXEC_NS = [None]
LAST_SCOPES = [None]


def kernel(**inputs) -> np.ndarray:
    nc = build_graph()
    in_maps = _prep(inputs)
    trace = bool(int(os.environ.get("KERNEL_TRACE", "0")))
    res = bass_utils.run_bass_kernel_spmd(
        nc, in_maps, list(range(8)), trace=trace,
        trace_cores=[0] if trace else None,
    )
    LAST_EXEC_NS[0] = res.exec_time_ns
    LAST_SCOPES[0] = res.per_core_scope_times
    if trace and res.instructions_and_trace:
        print("trace path:", res.instructions_and_trace[1])
    full = np.zeros((B, T, OUT), np.float32)
    for core in range(8):
        full[core // 2, _own_rows(core)] = res.results[core]["out"]
    return full



# revision 46
# speedup vs baseline: 1.2000x; 1.0042x over previous
"""Trainium2 Bass kernel for nn_AttentionDecoder_82738249990894 (B=4, T=1024,
C=1024, H=16, D=64, F=4096, L=4, vocab 64+1 outputs).

Sharding: sequence-split data parallel over 8 cores.  Core c handles batch
b = c//2, sequence half = c%2.  Balanced causal split: half0 owns global
128-row blocks [0,1,6,7], half1 owns [2,3,4,5] (equal attention work: both
see 18 causal k-tiles).  Per layer the pair exchanges rmsnorm'd activations
(bf16, pairwise AllGather, ~1MB) and each core recomputes k/v for all 1024
tokens locally.  No other communication.

SPMD uniformity: one graph runs on all 8 cores, so the key/value strip is
kept in GLOBAL token order (the AllGather return scatters both pair slots
to fixed global positions) and every local q-tile j computes scores against
the union visibility vis_u=[3,4,7,8] k-tiles; per-core 0/1 masks (input
data) encode causality and half-dependent visibility.

Matmul dtypes: fp8e4m3 with DoubleRow perf mode (2 k-tiles per pass, 2x
PE throughput) for the qkv/Wo/W1/W2/AV matmuls; weights are pre-scaled
(x64, x512 for Wq) into fp8's dynamic range on the host and the inverse
scale is folded into the psum->sbuf copy or activation that follows.
Scores stay bf16 (64-deep contraction, DoubleRow inapplicable).
Residual x stays fp32.  Softmax skips max-subtraction (scores are O(1);
fp32 psum exp is safe) and gets denominators free via a ones-column
appended to v; normalization is deferred to after the AV matmul.
"""
import os
import sys
import types

sys.path.insert(0, "/opt/trn_rl_repo")

import numpy as np
import ml_dtypes

import antenv

if not hasattr(antenv, "axon_hooks"):
    _mod = types.ModuleType("antenv.axon_hooks")
    _mod._hook = None
    _mod.set_axon_ntff_profile_hook = lambda h: setattr(_mod, "_hook", h)
    _mod.get_axon_ntff_profile_hook = lambda: _mod._hook
    sys.modules["antenv.axon_hooks"] = _mod
    antenv.axon_hooks = _mod
    try:
        from trn_agent_boot.trn_boot import _ntff_profile_via_ctypes

        _mod.set_axon_ntff_profile_hook(
            _ntff_profile_via_ctypes("/opt/axon/libaxon_pjrt.so")
        )
    except Exception:
        pass

import concourse.bass as bass
import concourse.mybir as mybir
import concourse.tile as tile
from concourse import bass_utils

bass_utils.upload_artifacts = lambda tmpdir: "local://" + tmpdir
try:
    from concourse import tile_utils as _tu

    _tu.max_sbuf_usage = 206 * 1024
except Exception:
    pass

F32 = mybir.dt.float32
F32R = mybir.dt.float32r
BF16 = mybir.dt.bfloat16
F8 = mybir.dt.float8e4
DR = mybir.MatmulPerfMode.DoubleRow
AF = mybir.ActivationFunctionType
OP = mybir.AluOpType
AX = mybir.AxisListType

SW = 64.0     # fp8 weight pre-scale (Wk/Wv/Wo/W1/W2)
SWQ = 512.0   # Wq pre-scale (D^-0.5 folded in makes it 8x smaller)

B, T, C, H, D, F, L = 4, 1024, 1024, 16, 64, 4096, 4
VOCAB, OUT = 64, 65
EPS = float(np.finfo(np.float32).eps)
RG = [[0, 1], [2, 3], [4, 5], [6, 7]]
OWN_BLOCKS = {0: [0, 1, 6, 7], 1: [2, 3, 4, 5]}
VIS_U = [3, 4, 7, 8]          # union visible k-tiles per local q-tile
N_MASK = 3                    # last 3 visible slots carry a mask

_wsplit_ctr = [0]


def _split_sync_waits(nc):
    """This walrus build allows one sync-wait per instruction; hoist extras
    onto injected same-engine NoOps."""
    for f in nc.m.functions:
        for bb in f.blocks:
            out = []
            changed = False
            for inst in bb.instructions:
                si = getattr(inst, "sync_info", None)
                if si is not None and si.on_wait is not None and len(si.on_wait) > 1:
                    waits = list(si.on_wait)
                    for w in waits[:-1]:
                        _wsplit_ctr[0] += 1
                        n = mybir.InstNoOp(
                            name=f"WSPLIT-{_wsplit_ctr[0]}", ins=[], outs=[]
                        )
                        n.engine = inst.engine
                        n.sync_info = mybir.SyncInfo(on_wait=[w], on_update=[])
                        out.append(n)
                    inst.sync_info = mybir.SyncInfo(
                        on_wait=[waits[-1]], on_update=list(si.on_update)
                    )
                    changed = True
                out.append(inst)
            if changed:
                bb.instructions[:] = out


def build_graph():
    nc = bass.Bass()
    dp = nc.declare_dram_parameter
    onehot_ext = dp("onehot_t", [OUT, 512], F32R, isOutput=False)
    pos_ext = dp("pos_fm", [128, 8, 512], BF16, isOutput=False)
    aug_ext = dp("aug_table", [OUT, 8, 128], F32R, isOutput=False)
    mask_ext = dp("masks", [128, 8, 512], BF16, isOutput=False)
    onescol_ext = dp("ones_col", [128, 1], BF16, isOutput=False)
    onesrow_ext = dp("ones_row", [1, 128], F32R, isOutput=False)
    onesrowb_ext = dp("ones_row_bf", [1, 128], BF16, isOutput=False)
    wq_ext = dp("Wq_arr", [L, 8, 128, 8, 128], F8, isOutput=False)
    wk_ext = dp("Wk_arr", [L, 8, 128, 8, 128], F8, isOutput=False)
    wv_ext = dp("Wv_arr", [L, 128, 2, 8, 512], F8, isOutput=False)
    wo_ext = dp("Wo_arr", [L, 8, 128, 8, 128], F8, isOutput=False)
    w1_ext = dp("W1_arr", [L, 32, 128, 8, 128], F8, isOutput=False)
    w2_ext = dp("W2_arr", [L, 16, 128, 2, 8, 128], F8, isOutput=False)
    bo_ext = dp("bo_fm", [128, L, 8, 1], F32, isOutput=False)
    b1_ext = dp("b1_fm", [128, L, 32, 1], F32, isOutput=False)
    b2_ext = dp("b2_fm", [128, L, 8, 1], F32, isOutput=False)
    lmw_ext = dp("lmW_arr", [128, 8, OUT], BF16, isOutput=False)
    lmb_ext = dp("lmb_bc", [128, OUT], F32, isOutput=False)
    out_ext = dp("out", [512, OUT], F32, isOutput=True)
    debug = bool(int(os.environ.get("KERNEL_DEBUG", "0")))
    if debug:
        dbg_h = dp("dbg_h", [128, 8, 1024], F8, isOutput=True)
        dbg_k = dp("dbg_k", [128, 8, 1024], BF16, isOutput=True)
        dbg_q = dp("dbg_q", [128, 8, 512], BF16, isOutput=True)
        dbg_v = dp("dbg_v", [128, 8, 16, 66], F8, isOutput=True)
        dbg_o = dp("dbg_o", [128, 8, 512], F8, isOutput=True)
        dbg_u = dp("dbg_u", [128, 4, 512], F8, isOutput=True)
        dbg_x2 = dp("dbg_x2", [128, 8, 512], F32, isOutput=True)

    with tile.TileContext(nc) as tc:
        nc_lp = nc.allow_low_precision(reason="bf16 attention path is intentional")
        nc_lp.__enter__()
        with (
            tc.tile_pool(name="persist", bufs=1) as pp,
            tc.tile_pool(name="scratch", bufs=2) as sp,
            tc.tile_pool(name="wqk", bufs=3) as wqkp,
            tc.tile_pool(name="w512", bufs=3) as w512p,
            tc.tile_pool(name="w2p", bufs=5) as w2p,
            tc.tile_pool(name="bigp", bufs=1) as bigp,
            tc.tile_pool(name="wvp", bufs=1) as wvp,
            tc.tile_pool(name="ps512", bufs=3, space="PSUM") as ps512,
            tc.tile_pool(name="ps128", bufs=2, space="PSUM") as ps128,
            tc.tile_pool(name="ps128o", bufs=3, space="PSUM") as ps128o,
            tc.tile_pool(name="dram", bufs=2, space="DRAM") as dram,
        ):
            # ---- constants ----
            ones_col = pp.tile([128, 1], BF16)
            ones_row = pp.tile([1, 128], F32R)
            ones_row_bf = pp.tile([1, 128], BF16)
            aug_sb = pp.tile([OUT, 8, 128], F32R)
            onehot_sb = pp.tile([OUT, 512], F32R)
            mask_sb = pp.tile([128, 8, 512], BF16)
            lmw_sb = pp.tile([128, 8, OUT], BF16)
            lmb_sb = pp.tile([128, OUT], F32)
            bo_sb = pp.tile([128, L, 8, 1], F32)
            b1_sb = pp.tile([128, L, 32, 1], F32)
            b2_sb = pp.tile([128, L, 8, 1], F32)
            nc.sync.dma_start(aug_sb[:], aug_ext[:])
            nc.sync.dma_start(onehot_sb[:], onehot_ext[:])
            nc.sync.dma_start(ones_col[:], onescol_ext[:])
            nc.sync.dma_start(ones_row[:], onesrow_ext[:])
            nc.sync.dma_start(ones_row_bf[:], onesrowb_ext[:])

            # tiny warm-up collective: absorbs first-use CC latency while
            # the tensor engine runs the embedding
            cw_in = dram.tile([1, 64], F32, tag="cwi", name="cwi")
            cw_out = dram.tile([2, 64], F32, tag="cwo", name="cwo")
            cw_sb = pp.tile([1, 64], F32)
            nc.gpsimd.memset(cw_sb[:], 0.0)
            nc.sync.dma_start(cw_in[:], cw_sb[:])
            nc.gpsimd.collective_compute(
                "AllGather", OP.bypass,
                ins=[cw_in[:].opt()],
                outs=[cw_out[:].opt()],
                replica_groups=RG,
            )

            nc.sync.dma_start(mask_sb[:], mask_ext[:])
            nc.sync.dma_start(lmw_sb[:], lmw_ext[:])
            nc.sync.dma_start(lmb_sb[:], lmb_ext[:])
            nc.sync.dma_start(bo_sb[:], bo_ext[:])
            nc.sync.dma_start(b1_sb[:], b1_ext[:])
            nc.sync.dma_start(b2_sb[:], b2_ext[:])

            eps_sb = pp.tile([128, 1], F32)
            nc.gpsimd.memset(eps_sb[:], EPS)

            # ---- persistent activations ----
            x_sb = pp.tile([128, 8, 512], F32)       # residual (feature-major)
            h_own = pp.tile([128, 8, 512], F8)       # norm'd own tokens
            h_str = pp.tile([128, 8, 1024], F8)      # norm'd pair, global order
            q_sb = pp.tile([128, 8, 512], BF16)      # [2h*64, hp, local t]
            k_sb = pp.tile([128, 8, 1024], BF16)     # [2h*64, hp, global t]
            # inner dim padded 65->66 so every fp8 slice lands on an even
            # byte offset (odd SBUF offsets break engine reads)
            v_sb = pp.tile([128, 8, 16, 66], F8)     # [tk, tkt, head, d+1+pad]
            o_sb = pp.tile([128, 8, 512], BF16)      # attn out, unnormalized
            o_f8 = pp.tile([128, 8, 512], F8)        # normalized attn out
            # pos and per-layer h2 share one big slot (disjoint lifetimes)
            pos_sb = bigp.tile([128, 8, 512], BF16, tag="big", name="pos")
            nc.sync.dma_start(pos_sb[:], pos_ext[:])

            # ---- embedding: x = onehot @ aug_table + pos ----
            sc_emb = nc.named_scope("emb"); sc_emb.__enter__()
            for ct in range(8):
                emb_ps = ps512.tile([128, 512], F32, tag="p5", name=f"emb{ct}")
                nc.tensor.matmul(emb_ps[:], aug_sb[:, ct, :], onehot_sb[:],
                                 start=True, stop=True)
                nc.vector.tensor_add(x_sb[:, ct, :], emb_ps[:], pos_sb[:, ct, :])

            sc_emb.__exit__(None, None, None)

            def rms_rbc(tag):
                ssum = ps512.tile([128, 512], F32, tag="p5", name=f"ss{tag}")
                for ct in range(8):
                    xsq = sp.tile([128, 512], BF16, tag="xsq", name=f"xq{tag}{ct}")
                    if ct % 2 == 0:
                        nc.scalar.activation(xsq[:], x_sb[:, ct, :], AF.Square)
                    else:
                        nc.vector.tensor_tensor(xsq[:], x_sb[:, ct, :],
                                                x_sb[:, ct, :], OP.mult)
                    nc.tensor.matmul(ssum[:1, :], ones_col[:], xsq[:],
                                     start=(ct == 0), stop=(ct == 7))
                lnv = sp.tile([1, 512], F32, tag="lnv", name=f"lv{tag}", bufs=1)
                nc.scalar.activation(lnv[:], ssum[:1, :], AF.Ln,
                                     bias=eps_sb[:1, :], scale=1.0 / C)
                rstd = sp.tile([1, 512], F32R, tag="sqv", name=f"sv{tag}",
                               bufs=1)
                nc.scalar.activation(rstd[:], lnv[:], AF.Exp, scale=-0.5)
                rbc = ps512.tile([128, 512], F32, tag="p5", name=f"rb{tag}")
                nc.tensor.matmul(rbc[:], ones_row[:], rstd[:], start=True,
                                 stop=True)
                return rbc

            for l in range(L):
                sc_n1 = nc.named_scope(f"n1.{l}"); sc_n1.__enter__()
                # ===== norm1 -> h_own =====
                rbc = rms_rbc(f"a{l}")
                for ct in range(8):
                    nc.vector.tensor_tensor(h_own[:, ct, :], x_sb[:, ct, :],
                                            rbc[:], OP.mult)

                # ===== pair exchange (AllGather, fp8) =====
                bounce = dram.tile([8, 128, 512], F8, tag="agin", name=f"agi{l}")
                for ct in range(8):
                    nc.sync.dma_start(bounce[ct], h_own[:, ct, :])
                gath = dram.tile([2, 8, 128, 512], F8, tag="agout",
                                 name=f"ago{l}")
                nc.gpsimd.collective_compute(
                    "AllGather", OP.bypass,
                    ins=[bounce[:].opt()],
                    outs=[gath[:].opt()],
                    replica_groups=RG,
                )

                sc_n1.__exit__(None, None, None)
                sc_q = nc.named_scope(f"q.{l}"); sc_q.__enter__()
                # Wv for this layer (no AG dependency -> overlaps exchange)
                wv_sb = wvp.tile([128, 2, 8, 512], F8, tag="wv", name=f"wv{l}")
                nc.sync.dma_start(wv_sb[:], wv_ext[l])

                # ===== q from h_own (overlaps AG) =====
                for hp in range(8):
                    wq_sb = wqkp.tile([128, 8, 128], F8, tag="wqk",
                                      name=f"wq{l}_{hp}")
                    nc.sync.dma_start(wq_sb[:], wq_ext[l, hp])
                    q_ps = ps512.tile([128, 512], F32, tag="p5", name=f"q{l}{hp}")
                    for cp in range(4):
                        nc.tensor.matmul(q_ps[:], wq_sb[:, 2 * cp:2 * cp + 2, :],
                                         h_own[:, 2 * cp:2 * cp + 2, :],
                                         start=(cp == 0), stop=(cp == 3),
                                         perf_mode=DR)
                    nc.vector.tensor_scalar_mul(q_sb[:, hp, :], q_ps[:],
                                                1.0 / SWQ)

                sc_q.__exit__(None, None, None)
                sc_kv = nc.named_scope(f"kv.{l}"); sc_kv.__enter__()
                # ===== scatter AG result into global-order strip =====
                # slot0 = half0 local blocks -> global [0,1,6,7]
                # slot1 = half1 local blocks -> global [2,3,4,5]
                for ct in range(8):
                    nc.sync.dma_start(h_str[:, ct, 0:256], gath[0, ct, :, 0:256])
                    nc.sync.dma_start(h_str[:, ct, 768:1024], gath[0, ct, :, 256:512])
                    nc.sync.dma_start(h_str[:, ct, 256:768], gath[1, ct])

                # ===== k over the strip =====
                for hp in range(8):
                    wk_sb = wqkp.tile([128, 8, 128], F8, tag="wqk",
                                      name=f"wk{l}_{hp}")
                    nc.sync.dma_start(wk_sb[:], wk_ext[l, hp])
                    for half in range(2):
                        k_pool, k_tag = ((ps512, "p5") if half == 0
                                         else (ps128, "pk"))
                        k_ps = k_pool.tile([128, 512], F32, tag=k_tag,
                                           name=f"k{l}{hp}{half}")
                        for cp in range(4):
                            nc.tensor.matmul(
                                k_ps[:], wk_sb[:, 2 * cp:2 * cp + 2, :],
                                h_str[:, 2 * cp:2 * cp + 2,
                                      half * 512:(half + 1) * 512],
                                start=(cp == 0), stop=(cp == 3), perf_mode=DR)
                        nc.vector.tensor_scalar_mul(
                            k_sb[:, hp, half * 512:(half + 1) * 512], k_ps[:],
                            1.0 / SW)

                sc_kv.__exit__(None, None, None)
                sc_at = nc.named_scope(f"at.{l}"); sc_at.__enter__()
                # ===== v tiles interleaved with per-head scores: the
                # scalar exp stream starts while the tensor engine is still
                # on v matmuls =====
                pending = []

                def _normalize(pend):
                    ph, pden = pend
                    php, poff = ph // 2, (ph % 2) * D
                    r = sp.tile([1, 512], BF16, tag="rex", name=f"re{l}_{ph}")
                    nc.scalar.activation(r[:], pden[:], AF.Exp, scale=-1.0)
                    rb_ps = ps128.tile([128, 512], F32, tag="pk",
                                       name=f"rb{l}_{ph}")
                    nc.tensor.matmul(rb_ps[poff:poff + D, :],
                                     ones_row_bf[:, 0:D],
                                     r[:], start=True, stop=True)
                    nc.vector.tensor_tensor(
                        o_f8[poff:poff + D, php, :], o_sb[poff:poff + D, php, :],
                        rb_ps[poff:poff + D, :], OP.mult)

                def v_tile(tkt):
                    for vh in range(2):
                        v_pool, v_tag = ((ps512, "p5") if vh == 0
                                         else (ps128, "pk"))
                        v_ps = v_pool.tile([128, 512], F32, tag=v_tag,
                                           name=f"v{l}{tkt}{vh}")
                        for cp in range(4):
                            nc.tensor.matmul(
                                v_ps[:],
                                h_str[:, 2 * cp:2 * cp + 2,
                                      tkt * 128:(tkt + 1) * 128],
                                wv_sb[:, vh, 2 * cp:2 * cp + 2, :],
                                start=(cp == 0), stop=(cp == 3), perf_mode=DR)
                        nc.vector.tensor_scalar_mul(
                            v_sb[:, tkt, 8 * vh:8 * vh + 8, 0:D],
                            v_ps[:].rearrange("p (q d) -> p q d", d=D),
                            1.0 / SW)
                    nc.gpsimd.memset(v_sb[:, tkt, :, D:66], 1.0)

                def head_scores(h16):
                    hp, off = h16 // 2, (h16 % 2) * D
                    ex = sp.tile([128, 8, 512], F8, tag="expA", bufs=3,
                                 name=f"ex{l}_{h16}")

                    def score_wide(s):
                        s_pool, s_tag = ((ps512, "p5") if s % 2 == 0
                                         else (ps128, "pk"))
                        s_ps = s_pool.tile([128, 512], F32, tag=s_tag,
                                           name=f"s{l}_{h16}_{s}")
                        nc.tensor.matmul(
                            s_ps[:],
                            k_sb[off:off + D, hp, s * 128:(s + 1) * 128],
                            q_sb[off:off + D, hp, :],
                            start=True, stop=True)
                        nc.scalar.activation(ex[:, s, :], s_ps[:], AF.Exp)

                    def score_narrow(pi):
                        s_pool, s_tag = ((ps512, "p5") if pi == 0
                                         else (ps128, "pk"))
                        n_ps = s_pool.tile([128, 512], F32, tag=s_tag,
                                           name=f"n{l}_{h16}_{pi}")
                        for j in range(2):
                            s = 4 + 2 * pi + j
                            nc.tensor.matmul(
                                n_ps[:, 256 * j:256 * (j + 1)],
                                k_sb[off:off + D, hp, s * 128:(s + 1) * 128],
                                q_sb[off:off + D, hp, 256:512],
                                start=True, stop=True)
                        s0 = 4 + 2 * pi
                        nc.scalar.activation(
                            ex[:, s0:s0 + 2, 256:512],
                            n_ps[:].rearrange("p (s m) -> p s m", s=2),
                            AF.Exp)

                    def mask2(s0, c0):
                        nc.vector.tensor_tensor(
                            ex[:, s0:s0 + 2, c0:c0 + 256],
                            ex[:, s0:s0 + 2, c0:c0 + 256],
                            mask_sb[:, s0:s0 + 2, c0:c0 + 256], OP.mult)

                    score_wide(0)
                    score_wide(1)
                    mask2(0, 0)
                    score_wide(2)
                    score_wide(3)
                    mask2(2, 0)
                    score_narrow(0)
                    mask2(4, 256)
                    score_narrow(1)
                    mask2(6, 256)
                    return ex

                def head_avs(h16, ex):
                    hp, off = h16 // 2, (h16 % 2) * D
                    o_ps = ps128o.tile([128, 512], F32, tag="po",
                                       name=f"o{l}_{h16}")

                    def av_pair(p):
                        s2 = 2 * p
                        vv = v_sb[:, s2:s2 + 2, h16, 0:OUT]
                        if p == 0:
                            nc.tensor.matmul(o_ps[:OUT, :], vv,
                                             ex[:, s2:s2 + 2, :],
                                             start=True, stop=False,
                                             perf_mode=DR)
                        elif p == 1:
                            nc.tensor.matmul(o_ps[:OUT, 0:256], vv,
                                             ex[:, s2:s2 + 2, 0:256],
                                             start=False, stop=True,
                                             perf_mode=DR)
                            nc.tensor.matmul(o_ps[:OUT, 256:512], vv,
                                             ex[:, s2:s2 + 2, 256:512],
                                             start=False, stop=False,
                                             perf_mode=DR)
                        else:
                            nc.tensor.matmul(o_ps[:OUT, 256:512], vv,
                                             ex[:, s2:s2 + 2, 256:512],
                                             start=False, stop=(p == 3),
                                             perf_mode=DR)

                    for p in range(4):
                        av_pair(p)
                    den = sp.tile([1, 512], F32, tag="rr", name=f"r{l}_{h16}")
                    nc.scalar.activation(den[:], o_ps[VOCAB:OUT, :], AF.Ln)
                    nc.vector.tensor_copy(o_sb[off:off + D, hp, :], o_ps[:D, :])
                    pending.append((h16, den))
                    while len(pending) > 1:
                        _normalize(pending.pop(0))

                for tkt in range(8):
                    v_tile(tkt)
                exs = {0: head_scores(0)}
                for h16 in range(16):
                    if h16 + 1 < 16:
                        exs[h16 + 1] = head_scores(h16 + 1)
                    head_avs(h16, exs.pop(h16))
                while pending:
                    _normalize(pending.pop(0))

                sc_at.__exit__(None, None, None)
                sc_wo = nc.named_scope(f"wo.{l}"); sc_wo.__enter__()
                # ===== Wo + residual =====
                for cot in range(8):
                    wo_sb = w512p.tile([128, 8, 128], F8, tag="w5",
                                       name=f"wo{l}_{cot}")
                    nc.sync.dma_start(wo_sb[:], wo_ext[l, cot])
                    xo_ps = ps512.tile([128, 512], F32, tag="p5",
                                       name=f"xo{l}{cot}")
                    for hdp in range(4):
                        nc.tensor.matmul(xo_ps[:],
                                         wo_sb[:, 2 * hdp:2 * hdp + 2, :],
                                         o_f8[:, 2 * hdp:2 * hdp + 2, :],
                                         start=(hdp == 0), stop=(hdp == 3),
                                         perf_mode=DR)
                    xo_sb = sp.tile([128, 512], F32, tag="xo", name=f"xs{l}{cot}")
                    nc.scalar.activation(xo_sb[:], xo_ps[:], AF.Identity,
                                         bias=bo_sb[:, l, cot, :],
                                         scale=1.0 / SW)
                    nc.vector.tensor_add(x_sb[:, cot, :], x_sb[:, cot, :],
                                         xo_sb[:])

                sc_wo.__exit__(None, None, None)
                sc_n2 = nc.named_scope(f"n2.{l}"); sc_n2.__enter__()
                # ===== norm2 -> h2 (fp8) =====
                h2_sb = bigp.tile([128, 8, 512], F8, tag="big", name=f"h2_{l}")
                rbc2 = rms_rbc(f"b{l}")
                for ct in range(8):
                    nc.vector.tensor_tensor(h2_sb[:, ct, :], x_sb[:, ct, :],
                                            rbc2[:], OP.mult)

                sc_n2.__exit__(None, None, None)
                sc_ff = nc.named_scope(f"ff.{l}"); sc_ff.__enter__()
                # ===== FFN (ft chunks of 4; fp8 DoubleRow both matmuls) =====
                def emit_w2(chunk, u_prev, w2c):
                    for cot in range(8):
                        y_ps = ps128o.tile([128, 512], F32, tag="po",
                                          name=f"y{l}{chunk}{cot}")
                        for p in range(2):
                            nc.tensor.matmul(y_ps[:], w2c[p][:, :, cot, :],
                                             u_prev[:, 2 * p:2 * p + 2, :],
                                             start=(p == 0), stop=(p == 1),
                                             perf_mode=DR)
                        nc.vector.scalar_tensor_tensor(
                            x_sb[:, cot, :], y_ps[:], 1.0 / SW,
                            x_sb[:, cot, :], OP.mult, OP.add)
                        if chunk == 0:
                            # b2 folded in early: runs on scalar while later
                            # chunks' matmuls stream, off the layer-end path
                            nc.scalar.add(x_sb[:, cot, :], x_sb[:, cot, :],
                                          b2_sb[:, l, cot, :])

                prev = None
                for chunk in range(8):
                    u_sb = sp.tile([128, 4, 512], F8, tag="u",
                                   name=f"u{l}_{chunk}")
                    w2c = []
                    for fi in range(4):
                        ft = chunk * 4 + fi
                        w1_sb = w512p.tile([128, 8, 128], F8, tag="w5",
                                           name=f"w1_{l}_{ft}")
                        nc.sync.dma_start(w1_sb[:], w1_ext[l, ft])
                        if fi % 2 == 0:
                            w2_sb = w2p.tile([128, 2, 8, 128], F8, tag="w2",
                                             name=f"w2_{l}_{chunk}_{fi // 2}")
                            nc.sync.dma_start(w2_sb[:],
                                              w2_ext[l, chunk * 2 + fi // 2])
                            w2c.append(w2_sb)
                        u_pool, u_tag = ((ps512, "p5") if fi % 2 == 0
                                         else (ps128, "pk"))
                        u_ps = u_pool.tile([128, 512], F32, tag=u_tag,
                                           name=f"u{l}{ft}")
                        for cp in range(4):
                            nc.tensor.matmul(u_ps[:],
                                             w1_sb[:, 2 * cp:2 * cp + 2, :],
                                             h2_sb[:, 2 * cp:2 * cp + 2, :],
                                             start=(cp == 0), stop=(cp == 3),
                                             perf_mode=DR)
                        nc.scalar.activation(u_sb[:, fi, :], u_ps[:], AF.Gelu,
                                             bias=b1_sb[:, l, ft, :],
                                             scale=1.0 / SW)
                    if prev is not None:
                        emit_w2(*prev)
                    prev = (chunk, u_sb, w2c)
                emit_w2(*prev)


                if debug and l == 0:
                    nc.sync.dma_start(dbg_h[:], h_str[:])
                    nc.sync.dma_start(dbg_k[:], k_sb[:])
                    nc.sync.dma_start(dbg_q[:], q_sb[:])
                    nc.sync.dma_start(dbg_v[:], v_sb[:])
                    nc.sync.dma_start(dbg_o[:], o_f8[:])
                    nc.sync.dma_start(dbg_u[:], u_sb[:])
                    nc.sync.dma_start(dbg_x2[:], x_sb[:])

                sc_ff.__exit__(None, None, None)

            # ===== lm head + log_softmax / log_sigmoid =====
            for tlt in range(4):
                lg = ps512.tile([128, OUT], F32, tag="p5", name=f"lg{tlt}")
                for ct in range(8):
                    xr = sp.tile([128, 128], BF16, tag="xr", name=f"xr{tlt}_{ct}")
                    nc.scalar.copy(xr[:], x_sb[:, ct, tlt * 128:(tlt + 1) * 128])
                    nc.tensor.matmul(lg[:], xr[:], lmw_sb[:, ct, :],
                                     start=(ct == 0), stop=(ct == 7))
                lgb = sp.tile([128, OUT], F32, tag="lgb", name=f"lgb{tlt}")
                nc.vector.tensor_add(lgb[:], lg[:], lmb_sb[:])
                m = sp.tile([128, 1], F32, tag="m", name=f"m{tlt}")
                nc.vector.reduce_max(m[:], lgb[:, 0:VOCAB], axis=AX.X)
                nm = sp.tile([128, 1], F32, tag="nm", name=f"nm{tlt}")
                nc.scalar.mul(nm[:], m[:], -1.0)
                e = sp.tile([128, VOCAB], F32, tag="e", name=f"e{tlt}")
                es = sp.tile([128, 1], F32, tag="es", name=f"es{tlt}")
                nc.scalar.activation(e[:], lgb[:, 0:VOCAB], AF.Exp, bias=nm[:],
                                     accum_out=es[:])
                lse = sp.tile([128, 1], F32, tag="lse", name=f"lse{tlt}")
                nc.scalar.activation(lse[:], es[:], AF.Ln)
                bt = sp.tile([128, 1], F32, tag="bt", name=f"bt{tlt}")
                nc.vector.tensor_tensor(bt[:], nm[:], lse[:], OP.subtract)
                outt = sp.tile([128, OUT], F32, tag="outt", name=f"ot{tlt}")
                nc.scalar.activation(outt[:, 0:VOCAB], lgb[:, 0:VOCAB],
                                     AF.Identity, bias=bt[:])
                sg = sp.tile([128, 1], F32, tag="sg", name=f"sg{tlt}")
                nc.scalar.activation(sg[:], lgb[:, VOCAB:OUT], AF.Sigmoid)
                nc.scalar.activation(outt[:, VOCAB:OUT], sg[:], AF.Ln)
                nc.sync.dma_start(out_ext[tlt * 128:(tlt + 1) * 128, :], outt[:])

    _split_sync_waits(nc)
    return nc


# ---------------------------------------------------------------------------
# host-side preparation
# ---------------------------------------------------------------------------
def _own_rows(core):
    return np.concatenate(
        [np.arange(b * 128, (b + 1) * 128) for b in OWN_BLOCKS[core % 2]]
    )


def _bf(a):
    return np.asarray(a, dtype=ml_dtypes.bfloat16)


def _f8(a, s):
    return np.clip(np.asarray(a, np.float32) * s, -240.0, 240.0).astype(
        ml_dtypes.float8_e4m3fn
    )


def _f32(a):
    return np.ascontiguousarray(a, dtype=np.float32)


def _prep(inputs):
    acts = np.asarray(inputs["acts"])
    durations = _f32(inputs["durations"])
    emb_table = _f32(inputs["emb_table"])
    pos_table = _f32(inputs["pos_table"])
    Wq, Wk, Wv = (_f32(inputs[k]) for k in ("Wq", "Wk", "Wv"))
    Wo, bo = _f32(inputs["Wo"]), _f32(inputs["bo"])
    W1, b1 = _f32(inputs["W1"]), _f32(inputs["b1"])
    W2, b2 = _f32(inputs["W2"]), _f32(inputs["b2"])
    g1, g2 = _f32(inputs["g1"]), _f32(inputs["g2"])
    lm_W, lm_b = _f32(inputs["lm_W"]), _f32(inputs["lm_b"])

    # fold g1 into Wq/Wk/Wv (q also gets the D^-0.5 score scale), g2 into W1
    Wq_eff = Wq * g1[:, None, :, None] * (D ** -0.5)
    Wk_eff = Wk * g1[:, None, :, None]
    Wv_eff = Wv * g1[:, None, :, None]
    W1_eff = W1 * g2[:, :, None]

    def qk_arr(A, s):  # [L,H,C,D] -> [L, hp, cp, ct, m], fp8 scaled by s
        A2 = A.transpose(0, 2, 1, 3).reshape(L, C, H * D)
        return _f8(A2.reshape(L, 8, 128, 8, 128).transpose(0, 3, 2, 1, 4), s)

    shared = {
        "aug_table": None, "ones_col": _bf(np.ones((128, 1))),
        "ones_row": _f32(np.ones((1, 128))),
        "ones_row_bf": _bf(np.ones((1, 128))),
        "Wq_arr": qk_arr(Wq_eff, SWQ), "Wk_arr": qk_arr(Wk_eff, SW),
        "Wv_arr": _f8(Wv_eff.transpose(0, 2, 1, 3).reshape(L, C, H * D)
                      .reshape(L, 8, 128, 2, 512).transpose(0, 2, 3, 1, 4), SW),
        "Wo_arr": _f8(Wo.reshape(L, 8, 128, 8, 128).transpose(0, 3, 2, 1, 4), SW),
        "W1_arr": _f8(W1_eff.reshape(L, 8, 128, 32, 128)
                      .transpose(0, 3, 2, 1, 4), SW),
        "W2_arr": _f8(W2.reshape(L, 16, 2, 128, 8, 128)
                      .transpose(0, 1, 3, 2, 4, 5), SW),
        "bo_fm": np.ascontiguousarray(
            bo.reshape(L, 8, 128).transpose(2, 0, 1))[..., None],
        "b1_fm": np.ascontiguousarray(
            b1.reshape(L, 32, 128).transpose(2, 0, 1))[..., None],
        "b2_fm": np.ascontiguousarray(
            b2.reshape(L, 8, 128).transpose(2, 0, 1))[..., None],
        "lmW_arr": _bf(lm_W.reshape(8, 128, OUT).transpose(1, 0, 2)),
        "lmb_bc": _f32(np.tile(lm_b[None, :], (128, 1))),
    }
    aug = np.zeros((OUT, C), np.float32)
    aug[:VOCAB, : C - 1] = emb_table
    aug[VOCAB, C - 1] = 1.0
    shared["aug_table"] = _f32(aug.reshape(OUT, 8, 128))

    in_maps = []
    for core in range(8):
        b, half = core // 2, core % 2
        rows = _own_rows(core)
        oh = np.zeros((OUT, 512), np.float32)
        oh[acts[b, rows], np.arange(512)] = 1.0
        oh[VOCAB, :] = durations[b, rows]
        pos = pos_table[rows].T.reshape(8, 128, 512).transpose(1, 0, 2)
        masks = np.ones((8, 128, 512), np.float32)
        for s in range(8):
            gk = s * 128
            ii = gk + np.arange(128)[:, None]
            for j in range(4):
                gq = OWN_BLOCKS[half][j] * 128
                jj = gq + np.arange(128)[None, :]
                masks[s, :, j * 128:(j + 1) * 128] = (ii <= jj)
        m = dict(shared)
        m["onehot_t"] = _f32(oh)
        m["pos_fm"] = _bf(pos)
        m["masks"] = _bf(masks.transpose(1, 0, 2))
        in_maps.append(m)
    return in_maps


LAST_EXEC_NS = [None]
LAST_SCOPES = [None]


def kernel(**inputs) -> np.ndarray:
    nc = build_graph()
    in_maps = _prep(inputs)
    trace = bool(int(os.environ.get("KERNEL_TRACE", "0")))
    res = bass_utils.run_bass_kernel_spmd(
        nc, in_maps, list(range(8)), trace=trace,
        trace_cores=[0] if trace else None,
    )
    LAST_EXEC_NS[0] = res.exec_time_ns
    LAST_SCOPES[0] = res.per_core_scope_times
    if trace and res.instructions_and_trace:
        print("trace path:", res.instructions_and_trace[1])
    full = np.zeros((B, T, OUT), np.float32)
    for core in range(8):
        full[core // 2, _own_rows(core)] = res.results[core]["out"]
    return full

